# revision 9
# baseline (speedup 1.0000x reference)
"""MultiHeadCrossAttention Trainium2 kernel (8 NeuronCores, SPMD).

Sharding: core c -> (n = c // 2, g = c % 2). Each core handles one query
batch n and half the heads (8 of 16, embed slice g*512:(g+1)*512).

Host side: transpose queries/keys/values into [dim, tokens] layout, compact
keys/values along KLEN by the per-n mask (~50% survive), pad to KC = 128*T,
cast to bf16. The device returns unnormalized AV (bf16) plus per-head
softmax denominator partials; the host divides while assembling.

Device side per core (all matmuls bf16, fp32 PSUM accumulation), built
around PE array tiling (tile_position) so half-size matmuls run
concurrently in disjoint array quadrants:
  - energy: per (head-pair c, q-chunk, k-tile) ONE slot runs BOTH heads'
    K=64 energies concurrently as 2x row-tiled matmuls (rows 0:63 = head0,
    64:127 = head1, matching the natural qT/kT embed layout).
  - exp on ScalarE (scale=1/8): one ACTIVATE per k-tile covering both
    heads' [128, 512] PSUM banks via a strided [128, 2, 512] read.
  - AV: per k-tile ONE slot runs both heads as 2x col-tiled matmuls
    (M=64 each) accumulating into one PSUM bank (h0 -> partitions 0:64,
    h1 -> 64:128; single start/stop pair for the whole bank).
  - softmax denominators: 4x col-tiled M=1 matmuls (lhsT = validity
    indicator column) covering (2 heads x 2 k-tiles) per slot, landing at
    PSUM partitions {0,32,64,96} of a shared bank.
  - projections stream 512 cols per matmul (v-projection done once for
    all head-pairs in [token, emb] layout); proj work for pair c+1 is
    interleaved into pair c's attention stream to fill PE stalls while
    ScalarE (the bottleneck) streams exp continuously.
"""

import math
import sys
from collections import deque
from contextlib import ExitStack

import numpy as np

for _p in ("/opt/trn_rl_repo",):
    if _p not in sys.path:
        sys.path.insert(0, _p)

import ml_dtypes

import concourse.bass as bass  # noqa: F401  (import registers lowering deps)
import concourse.tile as tile
from concourse import bacc, mybir
from concourse.bass_utils import run_bass_kernel_spmd

BF16 = ml_dtypes.bfloat16

N, QLEN, KLEN = 4, 2048, 2048
QDIM = KVDIM = 512
EMBED, HEADS = 1024, 16
HEAD_DIM = 64
N_CORES = 8
QCH = 512  # q-chunk width (one PSUM bank of fp32)
SCALE = 1.0 / math.sqrt(HEAD_DIM)

_cache: dict = {}
last_exec_time_ns = None
last_results = None


def _build(T: int, ql: int = QLEN):
    """Build the per-core Bass program for KC = 128*T compacted kv tokens."""
    KC = 128 * T
    dt = mybir.dt
    nc = bacc.Bacc("TRN2", target_bir_lowering=False, debug=False)

    qT_d = nc.dram_tensor("qt", [QDIM, ql], dt.bfloat16, kind="ExternalInput").ap()
    kT_d = nc.dram_tensor("kt", [KVDIM, KC], dt.bfloat16, kind="ExternalInput").ap()
    vT_d = nc.dram_tensor("vt", [KVDIM, KC], dt.bfloat16, kind="ExternalInput").ap()
    wq_d = nc.dram_tensor("wq", [QDIM, 512], dt.bfloat16, kind="ExternalInput").ap()
    wk_d = nc.dram_tensor("wk", [KVDIM, 512], dt.bfloat16, kind="ExternalInput").ap()
    wv_d = nc.dram_tensor("wv", [KVDIM, 512], dt.bfloat16, kind="ExternalInput").ap()
    # per-row validity indicator (1.0 real kv token, 0.0 pad), [128, T]
    vind_d = nc.dram_tensor("vind", [128, T], dt.bfloat16, kind="ExternalInput").ap()
    # unnormalized AV.T: rows c*128 + h*64 + d, cols q
    av_d = nc.dram_tensor("av", [512, ql], dt.bfloat16, kind="ExternalOutput").ap()
    # denominator partials: rows c*128 + {0,32,64,96} = (h, k-tile parity)
    den_d = nc.dram_tensor("den", [512, ql], dt.bfloat16, kind="ExternalOutput").ap()

    NQ = ql // QCH
    kcols = [(s, min(512, KC - s)) for s in range(0, KC, 512)]

    with tile.TileContext(nc) as tc:
        with ExitStack() as ctx:
            persist = ctx.enter_context(tc.tile_pool(name="persist", bufs=1))

            qTin = [persist.tile([128, ql], dt.bfloat16, tag=f"qTin{j}", name=f"qTin{j}") for j in range(4)]
            kTin = [persist.tile([128, KC], dt.bfloat16, tag=f"kTin{j}", name=f"kTin{j}") for j in range(4)]
            vTin = [persist.tile([128, KC], dt.bfloat16, tag=f"vTin{j}", name=f"vTin{j}") for j in range(4)]
            wsb = {
                nm: [persist.tile([128, 512], dt.bfloat16, tag=f"{nm}{j}", name=f"{nm}{j}") for j in range(4)]
                for nm in ("wq", "wk", "wv")
            }
            qT = [persist.tile([128, ql], dt.bfloat16, tag=f"qT{c}", name=f"qT{c}") for c in range(4)]
            kT = [persist.tile([128, KC], dt.bfloat16, tag=f"kT{c}", name=f"kT{c}") for c in range(4)]
            # v in [token, emb-within-g] layout: AV lhsT for (c, h) is
            # vsb[:, t, c*128+h*64 : +64]
            vsb = persist.tile([128, T, 512], dt.bfloat16, tag="v", name="v")
            vind = persist.tile([128, T], dt.bfloat16, tag="vind", name="vind")
            junk = persist.tile([128, 512], dt.bfloat16, tag="junk", name="junk")

            # DMA order: k-side first so the first projections can start early
            for j in range(4):
                nc.sync.dma_start(wsb["wk"][j], wk_d[j * 128:(j + 1) * 128, :])
            for j in range(4):
                nc.sync.dma_start(kTin[j], kT_d[j * 128:(j + 1) * 128, :])
            for j in range(4):
                nc.sync.dma_start(wsb["wq"][j], wq_d[j * 128:(j + 1) * 128, :])
            for j in range(4):
                nc.sync.dma_start(qTin[j], qT_d[j * 128:(j + 1) * 128, :])
            for j in range(4):
                nc.sync.dma_start(wsb["wv"][j], wv_d[j * 128:(j + 1) * 128, :])
            for j in range(4):
                nc.sync.dma_start(vTin[j], vT_d[j * 128:(j + 1) * 128, :])
            nc.sync.dma_start(vind, vind_d)
            nc.vector.memset(junk, 1.0)

            with tc.tile_pool(name="psA", bufs=2, space="PSUM") as psA, \
                 tc.tile_pool(name="psE", bufs=2, space="PSUM") as psE, \
                 tc.tile_pool(name="psO", bufs=1, space="PSUM") as psO, \
                 tc.tile_pool(name="sbx", bufs=4) as sbx, \
                 tc.tile_pool(name="sbo", bufs=2) as sbo:

                # PE clock warm-up during the input-DMA window
                for _ in range(3):
                    ps = psA.tile([128, QCH], dt.float32, tag="pA", name="pA")
                    for r in range(10):
                        nc.tensor.matmul(ps, lhsT=junk[:, :128], rhs=junk,
                                         start=(r == 0), stop=(r == 9))

                # ---- projection emitters ----
                def emit_kproj(c, s, w):
                    ps = psA.tile([128, QCH], dt.float32, tag="pA", name="pA")
                    for j in range(4):
                        nc.tensor.matmul(
                            ps[:, :w],
                            lhsT=wsb["wk"][j][:, c * 128:(c + 1) * 128],
                            rhs=kTin[j][:, s:s + w],
                            start=(j == 0), stop=(j == 3),
                        )
                    nc.vector.tensor_copy(kT[c][:, s:s + w], ps[:, :w])

                def emit_qproj(c, q0):
                    ps = psA.tile([128, QCH], dt.float32, tag="pA", name="pA")
                    for j in range(4):
                        nc.tensor.matmul(
                            ps,
                            lhsT=wsb["wq"][j][:, c * 128:(c + 1) * 128],
                            rhs=qTin[j][:, q0 * QCH:(q0 + 1) * QCH],
                            start=(j == 0), stop=(j == 3),
                        )
                    nc.vector.tensor_copy(qT[c][:, q0 * QCH:(q0 + 1) * QCH], ps)

                def emit_vproj(t):
                    ps = psA.tile([128, QCH], dt.float32, tag="pA", name="pA")
                    for j in range(4):
                        nc.tensor.matmul(
                            ps,
                            lhsT=vTin[j][:, t * 128:(t + 1) * 128],
                            rhs=wsb["wv"][j],
                            start=(j == 0), stop=(j == 3),
                        )
                    nc.vector.tensor_copy(vsb[:, t, :], ps)

                # ---- attention emitters ----
                def emit_E(c, q0, t):
                    # both heads' K=64 energies concurrently (2x row tiling)
                    eb = psE.tile([128, 2, QCH], dt.float32, tag="e", name="e")
                    for h in range(2):
                        nc.tensor.matmul(
                            eb[:, h, :],
                            lhsT=kT[c][h * 64:(h + 1) * 64, t * 128:(t + 1) * 128],
                            rhs=qT[c][h * 64:(h + 1) * 64, q0 * QCH:(q0 + 1) * QCH],
                            start=True, stop=True,
                        )
                    return eb

                def emit_X(eb):
                    ex = sbx.tile([128, 2, QCH], dt.bfloat16, tag="x", name="x")
                    nc.scalar.activation(
                        ex, eb, mybir.ActivationFunctionType.Exp, scale=SCALE,
                    )
                    return ex

                def emit_A(c, t, ex, av):
                    # both heads' M=64 AV concurrently (2x col tiling);
                    # has_written is per-element on HW, so each col-tile
                    # region carries its own start/stop
                    for h in range(2):
                        nc.tensor.matmul(
                            av[h * 64:(h + 1) * 64, :],
                            lhsT=vsb[:, t, c * 128 + h * 64:c * 128 + (h + 1) * 64],
                            rhs=ex[:, h, :],
                            start=(t == 0), stop=(t == T - 1),
                            skip_group_check=True,
                        )

                # last k-tile using each parity (for per-region stop flags)
                t_last = {0: ((T - 1) // 2) * 2, 1: ((T - 2) // 2) * 2 + 1 if T > 1 else None}

                def emit_D(pairs, den):
                    # 4x col-tiled M=1 denominator partials at partitions
                    # 64*(t%2) + 32*h of the shared bank; per-region
                    # start/stop (has_written is per-element on HW)
                    for (h, t, ex_t) in pairs:
                        pos = 64 * (t % 2) + 32 * h
                        nc.tensor.matmul(
                            den[pos:pos + 1, :],
                            lhsT=vind[:, t:t + 1],
                            rhs=ex_t[:, h, :],
                            start=(t < 2), stop=(t == t_last[t % 2]),
                            skip_group_check=True,
                            tile_position=(0, pos),
                        )

                def emit_out(c, q0, av, den):
                    oav = sbo.tile([128, QCH], dt.bfloat16, tag="oav", name="oav")
                    nc.vector.tensor_copy(oav, av)
                    nc.sync.dma_start(
                        av_d[c * 128:(c + 1) * 128, q0 * QCH:(q0 + 1) * QCH], oav)
                    oden = sbo.tile([97, QCH], dt.bfloat16, tag="oden", name="oden")
                    nc.vector.tensor_copy(oden, den[0:97, :])
                    nc.sync.dma_start(
                        den_d[c * 128:c * 128 + 97, q0 * QCH:(q0 + 1) * QCH], oden)

                # ---- side-task queues: projections interleaved into the
                # attention stream (they fill PE stalls during exp waits) ----
                side = {c: deque() for c in range(4)}
                side[0].extend([(emit_vproj, (t,)) for t in range(2, T)])
                side[0].extend([(emit_qproj, (0, q0)) for q0 in (1, 2, 3)])
                for c in range(1, 4):
                    side[c].extend([(emit_kproj, (c, s, w)) for (s, w) in kcols])
                    side[c].extend([(emit_qproj, (c, q0)) for q0 in range(NQ)])

                # prefix: minimum projections for the first attention items
                for (s, w) in kcols:
                    emit_kproj(0, s, w)
                emit_qproj(0, 0)
                emit_vproj(0)
                if T > 1:
                    emit_vproj(1)

                # ---- software-pipelined attention stream ----
                stream = [(c, q0, t) for c in range(4) for q0 in range(NQ)
                          for t in range(T)]

                state = {}  # (c, q0) -> dict(av=, den=, ex={t: tile}, dfirst=)

                def finish(item, eb):
                    c, q0, t = item
                    st = state[(c, q0)]
                    # side task first: it sits in the PE queue before the
                    # exp-gated AV matmuls, filling the stall window; drain
                    # this pair's leftovers, then the NEXT pair's projections
                    sq = None
                    if side[c]:
                        sq = side[c]
                    elif c + 1 < 4 and side[c + 1]:
                        sq = side[c + 1]
                    if sq:
                        fn, args = sq.popleft()
                        fn(*args)
                    ex = emit_X(eb)
                    st["ex"][t] = ex
                    emit_A(c, t, ex, st["av"])
                    if t % 2 == 1 or t == T - 1:
                        pairs = []
                        for tt in ([t - 1, t] if t % 2 == 1 else [t]):
                            for h in range(2):
                                pairs.append((h, tt, st["ex"][tt]))
                        emit_D(pairs, st["den"])
                        for tt in set(tt for _, tt, _ in pairs):
                            st["ex"].pop(tt, None)
                    if t == T - 1:
                        emit_out(c, q0, st["av"], st["den"])
                        del state[(c, q0)]

                prev = None
                for item in stream:
                    c, q0, t = item
                    if t == 0:
                        state[(c, q0)] = {
                            "av": psO.tile([128, QCH], dt.float32, tag="av", name="av"),
                            "den": psO.tile([128, QCH], dt.float32, tag="den", name="den"),
                            "ex": {},
                        }
                    eb = emit_E(c, q0, t)
                    if prev is not None:
                        finish(*prev)
                    prev = (item, eb)
                finish(*prev)

    nc.compile()
    return nc


def _prepare(queries, keys, values, mask):
    """Host-side sharding: transpose, compact kv by mask, validity tiles."""
    m = np.asarray(mask).reshape(N, KLEN) != 0
    idx = [np.nonzero(m[n])[0] for n in range(N)]
    cnts = [len(i) for i in idx]
    T = max(1, (max(cnts) + 127) // 128)
    KC = 128 * T

    kT_full = np.ascontiguousarray(np.asarray(keys, np.float32)[0].T)
    vT_full = np.ascontiguousarray(np.asarray(values, np.float32)[0].T)
    q32 = np.asarray(queries, np.float32)

    qT_n, kT_n, vT_n, vind_n = [], [], [], []
    for n in range(N):
        kt = np.zeros((KVDIM, KC), np.float32)
        vt = np.zeros((KVDIM, KC), np.float32)
        kt[:, :cnts[n]] = kT_full[:, idx[n]]
        vt[:, :cnts[n]] = vT_full[:, idx[n]]
        ind = (np.arange(KC) < cnts[n]).astype(np.float32)
        vind_n.append(np.ascontiguousarray(ind.reshape(T, 128).T).astype(BF16))
        kT_n.append(kt.astype(BF16))
        vT_n.append(vt.astype(BF16))
        qT_n.append(np.ascontiguousarray(q32[n].T).astype(BF16))
    return T, qT_n, kT_n, vT_n, vind_n


def kernel(queries, keys, values, mask, Wq, Wk, Wv, _trace=False):
    global last_exec_time_ns, last_results
    T, qT_n, kT_n, vT_n, vind_n = _prepare(queries, keys, values, mask)

    w_g = {}
    for nm, W in (("wq", Wq), ("wk", Wk), ("wv", Wv)):
        W = np.asarray(W, np.float32)
        w_g[nm] = [np.ascontiguousarray(W[:, g * 512:(g + 1) * 512]).astype(BF16)
                   for g in range(2)]

    nc = _cache.get(T)
    if nc is None:
        nc = _cache.setdefault(T, _build(T))

    in_maps = []
    for core in range(N_CORES):
        n, g = core // 2, core % 2
        in_maps.append({
            "qt": qT_n[n], "kt": kT_n[n], "vt": vT_n[n],
            "wq": w_g["wq"][g], "wk": w_g["wk"][g], "wv": w_g["wv"][g],
            "vind": vind_n[n],
        })

    res = run_bass_kernel_spmd(nc, in_maps, core_ids=list(range(N_CORES)),
                               trace=bool(_trace))
    last_exec_time_ns = res.exec_time_ns
    last_results = res

    full = np.empty((N, QLEN, EMBED), np.float32)
    for core in range(N_CORES):
        n, g = core // 2, core % 2
        av = np.asarray(res.results[core]["av"], dtype=np.float32)
        den = np.asarray(res.results[core]["den"], dtype=np.float32)
        av = av.reshape(4, 2, 64, QLEN)                  # [c, h, d, q]
        den = den.reshape(4, 128, QLEN)
        denom = np.stack([den[:, 0, :] + den[:, 64, :],
                          den[:, 32, :] + den[:, 96, :]], axis=1)  # [c, h, q]
        vals = av / denom[:, :, None, :]                 # [c, h, d, q]
        full[n, :, g * 512:(g + 1) * 512] = (
            vals.reshape(512, QLEN).T
        )
    return full


# revision 16
# speedup vs baseline: 1.0643x; 1.0643x over previous
"""MultiHeadCrossAttention Trainium2 kernel (8 NeuronCores, SPMD).

Sharding: core c -> (n = c // 2, g = c % 2). Each core handles one query
batch n and half the heads (8 of 16, embed slice g*512:(g+1)*512).

Host side: transpose queries/keys/values into [dim, tokens] layout, compact
keys/values along KLEN by the per-n mask (~50% survive), pad to KC = 128*T,
cast to bf16. The device returns unnormalized AV (bf16) plus per-head
softmax denominator partials; the host divides while assembling.

Device side per core (all matmuls bf16, fp32 PSUM accumulation), built
around PE array tiling (tile_position) so half-size matmuls run
concurrently in disjoint array quadrants:
  - energy: per (head-pair c, q-chunk, k-tile) ONE slot runs BOTH heads'
    K=64 energies concurrently as 2x row-tiled matmuls (rows 0:63 = head0,
    64:127 = head1, matching the natural qT/kT embed layout).
  - exp on ScalarE (scale=1/8): one ACTIVATE per k-tile covering both
    heads' [128, 512] PSUM banks via a strided [128, 2, 512] read.
  - AV: per k-tile ONE slot runs both heads as 2x col-tiled matmuls
    (M=64 each) accumulating into one PSUM bank (h0 -> partitions 0:64,
    h1 -> 64:128; single start/stop pair for the whole bank).
  - softmax denominators: 4x col-tiled M=1 matmuls (lhsT = validity
    indicator column) covering (2 heads x 2 k-tiles) per slot, landing at
    PSUM partitions {0,32,64,96} of a shared bank.
  - projections stream 512 cols per matmul (v-projection done once for
    all head-pairs in [token, emb] layout); proj work for pair c+1 is
    interleaved into pair c's attention stream to fill PE stalls while
    ScalarE (the bottleneck) streams exp continuously.
"""

import math
import sys
from collections import deque
from contextlib import ExitStack

import numpy as np

for _p in ("/opt/trn_rl_repo",):
    if _p not in sys.path:
        sys.path.insert(0, _p)

import ml_dtypes

import concourse.bass as bass  # noqa: F401  (import registers lowering deps)
import concourse.tile as tile
from concourse import bacc, mybir
from concourse.bass_utils import run_bass_kernel_spmd

BF16 = ml_dtypes.bfloat16

N, QLEN, KLEN = 4, 2048, 2048
QDIM = KVDIM = 512
EMBED, HEADS = 1024, 16
HEAD_DIM = 64
N_CORES = 8
QCH = 512  # q-chunk width (one PSUM bank of fp32)
SCALE = 1.0 / math.sqrt(HEAD_DIM)

_cache: dict = {}
last_exec_time_ns = None
last_results = None


def _build(T: int, ql: int = QLEN):
    """Build the per-core Bass program for KC = 128*T compacted kv tokens."""
    KC = 128 * T
    dt = mybir.dt
    nc = bacc.Bacc("TRN2", target_bir_lowering=False, debug=False)

    qT_d = nc.dram_tensor("qt", [QDIM, ql], dt.bfloat16, kind="ExternalInput").ap()
    kT_d = nc.dram_tensor("kt", [KVDIM, KC], dt.bfloat16, kind="ExternalInput").ap()
    vT_d = nc.dram_tensor("vt", [KVDIM, KC], dt.bfloat16, kind="ExternalInput").ap()
    wq_d = nc.dram_tensor("wq", [QDIM, 512], dt.bfloat16, kind="ExternalInput").ap()
    wk_d = nc.dram_tensor("wk", [KVDIM, 512], dt.bfloat16, kind="ExternalInput").ap()
    wv_d = nc.dram_tensor("wv", [KVDIM, 512], dt.bfloat16, kind="ExternalInput").ap()
    # per-row validity indicator (1.0 real kv token, 0.0 pad), [128, T]
    vind_d = nc.dram_tensor("vind", [128, T], dt.bfloat16, kind="ExternalInput").ap()
    # unnormalized AV.T: rows c*128 + h*64 + d, cols q
    av_d = nc.dram_tensor("av", [512, ql], dt.bfloat16, kind="ExternalOutput").ap()
    # denominator partials: rows c*128 + {0,32,64,96} = (h, k-tile parity)
    den_d = nc.dram_tensor("den", [512, ql], dt.bfloat16, kind="ExternalOutput").ap()

    NQ = ql // QCH
    kcols = [(s, min(512, KC - s)) for s in range(0, KC, 512)]

    with tile.TileContext(nc) as tc:
        with ExitStack() as ctx:
            persist = ctx.enter_context(tc.tile_pool(name="persist", bufs=1))

            qTin = [persist.tile([128, ql], dt.bfloat16, tag=f"qTin{j}", name=f"qTin{j}") for j in range(4)]
            kTin = [persist.tile([128, KC], dt.bfloat16, tag=f"kTin{j}", name=f"kTin{j}") for j in range(4)]
            vTin = [persist.tile([128, KC], dt.bfloat16, tag=f"vTin{j}", name=f"vTin{j}") for j in range(4)]
            wsb = {
                nm: [persist.tile([128, 512], dt.bfloat16, tag=f"{nm}{j}", name=f"{nm}{j}") for j in range(4)]
                for nm in ("wq", "wk", "wv")
            }
            qT = [persist.tile([128, ql], dt.bfloat16, tag=f"qT{c}", name=f"qT{c}") for c in range(4)]
            kT = [persist.tile([128, KC], dt.bfloat16, tag=f"kT{c}", name=f"kT{c}") for c in range(4)]
            # v in [token, emb-within-g] layout: AV lhsT for (c, h) is
            # vsb[:, t, c*128+h*64 : +64]
            vsb = persist.tile([128, T, 512], dt.bfloat16, tag="v", name="v")
            vind = persist.tile([128, T], dt.bfloat16, tag="vind", name="vind")
            junk = persist.tile([128, 512], dt.bfloat16, tag="junk", name="junk")

            # DMA order: k-side first so the first projections can start early
            for j in range(4):
                nc.sync.dma_start(wsb["wk"][j], wk_d[j * 128:(j + 1) * 128, :])
            for j in range(4):
                nc.sync.dma_start(kTin[j], kT_d[j * 128:(j + 1) * 128, :])
            for j in range(4):
                nc.sync.dma_start(wsb["wq"][j], wq_d[j * 128:(j + 1) * 128, :])
            for j in range(4):
                nc.sync.dma_start(qTin[j], qT_d[j * 128:(j + 1) * 128, :])
            for j in range(4):
                nc.sync.dma_start(wsb["wv"][j], wv_d[j * 128:(j + 1) * 128, :])
            for j in range(4):
                nc.sync.dma_start(vTin[j], vT_d[j * 128:(j + 1) * 128, :])
            nc.sync.dma_start(vind, vind_d)
            nc.vector.memset(junk, 1.0)

            with tc.tile_pool(name="psA", bufs=2, space="PSUM") as psA, \
                 tc.tile_pool(name="psE", bufs=2, space="PSUM") as psE, \
                 tc.tile_pool(name="psO", bufs=1, space="PSUM") as psO, \
                 tc.tile_pool(name="sbx", bufs=4) as sbx, \
                 tc.tile_pool(name="sbo", bufs=2) as sbo:

                # PE clock warm-up during the input-DMA window
                for _ in range(3):
                    ps = psA.tile([128, QCH], dt.float32, tag="pA", name="pA")
                    for r in range(10):
                        nc.tensor.matmul(ps, lhsT=junk[:, :128], rhs=junk,
                                         start=(r == 0), stop=(r == 9))

                # ---- projection emitters ----
                def emit_kproj(c, s, w):
                    ps = psA.tile([128, QCH], dt.float32, tag="pA", name="pA")
                    for j in range(4):
                        nc.tensor.matmul(
                            ps[:, :w],
                            lhsT=wsb["wk"][j][:, c * 128:(c + 1) * 128],
                            rhs=kTin[j][:, s:s + w],
                            start=(j == 0), stop=(j == 3),
                        )
                    nc.vector.tensor_copy(kT[c][:, s:s + w], ps[:, :w])

                def emit_qproj(c, q0):
                    ps = psA.tile([128, QCH], dt.float32, tag="pA", name="pA")
                    for j in range(4):
                        nc.tensor.matmul(
                            ps,
                            lhsT=wsb["wq"][j][:, c * 128:(c + 1) * 128],
                            rhs=qTin[j][:, q0 * QCH:(q0 + 1) * QCH],
                            start=(j == 0), stop=(j == 3),
                        )
                    nc.vector.tensor_copy(qT[c][:, q0 * QCH:(q0 + 1) * QCH], ps)

                def emit_vproj(t):
                    ps = psA.tile([128, QCH], dt.float32, tag="pA", name="pA")
                    for j in range(4):
                        nc.tensor.matmul(
                            ps,
                            lhsT=vTin[j][:, t * 128:(t + 1) * 128],
                            rhs=wsb["wv"][j],
                            start=(j == 0), stop=(j == 3),
                        )
                    nc.vector.tensor_copy(vsb[:, t, :], ps)

                # ---- attention emitters ----
                def emit_E(c, q0, t):
                    # both heads' K=64 energies concurrently (2x row tiling);
                    # flat [128, 1024] tile (contiguous 2 banks) so the exp
                    # ACTIVATE reads a single flat AP
                    eb = psE.tile([128, 2 * QCH], dt.float32, tag="e", name="e")
                    for h in range(2):
                        nc.tensor.matmul(
                            eb[:, h * QCH:(h + 1) * QCH],
                            lhsT=kT[c][h * 64:(h + 1) * 64, t * 128:(t + 1) * 128],
                            rhs=qT[c][h * 64:(h + 1) * 64, q0 * QCH:(q0 + 1) * QCH],
                            start=True, stop=True,
                        )
                    return eb

                def emit_X(eb):
                    ex = sbx.tile([128, 2 * QCH], dt.bfloat16, tag="x", name="x")
                    nc.scalar.activation(
                        ex, eb, mybir.ActivationFunctionType.Exp, scale=SCALE,
                    )
                    return ex

                def emit_A(c, t, ex, av):
                    # both heads' M=64 AV concurrently (2x col tiling);
                    # has_written is per-element on HW, so each col-tile
                    # region carries its own start/stop
                    for h in range(2):
                        nc.tensor.matmul(
                            av[h * 64:(h + 1) * 64, :],
                            lhsT=vsb[:, t, c * 128 + h * 64:c * 128 + (h + 1) * 64],
                            rhs=ex[:, h * QCH:(h + 1) * QCH],
                            start=(t == 0), stop=(t == T - 1),
                            skip_group_check=True,
                        )

                # last k-tile using each parity (for per-region stop flags)
                t_last = {0: ((T - 1) // 2) * 2, 1: ((T - 2) // 2) * 2 + 1 if T > 1 else None}

                def emit_D(pairs, den):
                    # 4x col-tiled M=1 denominator partials at partitions
                    # 64*(t%2) + 32*h of the shared bank; per-region
                    # start/stop (has_written is per-element on HW)
                    for (h, t, ex_t) in pairs:
                        pos = 64 * (t % 2) + 32 * h
                        nc.tensor.matmul(
                            den[pos:pos + 1, :],
                            lhsT=vind[:, t:t + 1],
                            rhs=ex_t[:, h * QCH:(h + 1) * QCH],
                            start=(t < 2), stop=(t == t_last[t % 2]),
                            skip_group_check=True,
                            tile_position=(0, pos),
                        )

                def emit_out(c, q0, av, den):
                    oav = sbo.tile([128, QCH], dt.bfloat16, tag="oav", name="oav")
                    nc.vector.tensor_copy(oav, av)
                    nc.sync.dma_start(
                        av_d[c * 128:(c + 1) * 128, q0 * QCH:(q0 + 1) * QCH], oav)
                    oden = sbo.tile([97, QCH], dt.bfloat16, tag="oden", name="oden")
                    nc.vector.tensor_copy(oden, den[0:97, :])
                    nc.sync.dma_start(
                        den_d[c * 128:c * 128 + 97, q0 * QCH:(q0 + 1) * QCH], oden)

                # ---- side-task queues: projections interleaved into the
                # attention stream (they fill PE stalls during exp waits) ----
                side = {c: deque() for c in range(4)}
                side[0].extend([(emit_vproj, (t,)) for t in range(2, T)])
                side[0].extend([(emit_qproj, (0, q0)) for q0 in (1, 2, 3)])
                for c in range(1, 4):
                    side[c].extend([(emit_kproj, (c, s, w)) for (s, w) in kcols])
                    side[c].extend([(emit_qproj, (c, q0)) for q0 in range(NQ)])

                # prefix: minimum projections for the first attention items
                for (s, w) in kcols:
                    emit_kproj(0, s, w)
                emit_qproj(0, 0)
                emit_vproj(0)
                if T > 1:
                    emit_vproj(1)

                # ---- software-pipelined attention stream ----
                stream = [(c, q0, t) for c in range(4) for q0 in range(NQ)
                          for t in range(T)]

                state = {}  # (c, q0) -> dict(av=, den=, ex={t: tile}, dfirst=)

                def do_X(item, eb):
                    c, q0, t = item
                    st = state[(c, q0)]
                    ex = emit_X(eb)
                    st["ex"][t] = ex

                def do_A(item):
                    c, q0, t = item
                    st = state[(c, q0)]
                    emit_A(c, t, st["ex"][t], st["av"])
                    if t % 2 == 1 or t == T - 1:
                        pairs = []
                        for tt in ([t - 1, t] if t % 2 == 1 else [t]):
                            for h in range(2):
                                pairs.append((h, tt, st["ex"][tt]))
                        emit_D(pairs, st["den"])
                        for tt in set(tt for _, tt, _ in pairs):
                            st["ex"].pop(tt, None)
                    if t == T - 1:
                        emit_out(c, q0, st["av"], st["den"])
                        del state[(c, q0)]

                # software pipeline, per iteration j: E(j), X(j-1), A(j-2).
                # X(j-1) is emitted before E(j+1) (the next writer of its
                # psE buffer, bufs=2) so the WAR is tracked; every emitted
                # instruction's dependencies completed >= 1 full period ago,
                # so neither engine ever stalls mid-stream.
                pend_x = deque()
                pend_a = deque()
                for item in stream:
                    c, q0, t = item
                    if t == 0:
                        state[(c, q0)] = {
                            "av": psO.tile([128, QCH], dt.float32, tag="av", name="av"),
                            "den": psO.tile([128, QCH], dt.float32, tag="den", name="den"),
                            "ex": {},
                        }
                    eb = emit_E(c, q0, t)
                    # side projection task right after the energy pair: it
                    # fills the PE queue without gating anything downstream,
                    # and runs 2+ iterations before its outputs are consumed
                    sq = None
                    if side[c]:
                        sq = side[c]
                    elif c + 1 < 4 and side[c + 1]:
                        sq = side[c + 1]
                    if sq:
                        fn, args = sq.popleft()
                        fn(*args)
                    pend_x.append((item, eb))
                    if len(pend_x) >= 2:
                        it2, eb2 = pend_x.popleft()
                        do_X(it2, eb2)
                        pend_a.append(it2)
                    if len(pend_a) >= 2:
                        do_A(pend_a.popleft())
                while pend_x:
                    it2, eb2 = pend_x.popleft()
                    do_X(it2, eb2)
                    pend_a.append(it2)
                while pend_a:
                    do_A(pend_a.popleft())

    nc.compile()
    return nc


def _prepare(queries, keys, values, mask):
    """Host-side sharding: transpose, compact kv by mask, validity tiles."""
    m = np.asarray(mask).reshape(N, KLEN) != 0
    idx = [np.nonzero(m[n])[0] for n in range(N)]
    cnts = [len(i) for i in idx]
    T = max(1, (max(cnts) + 127) // 128)
    KC = 128 * T

    kT_full = np.ascontiguousarray(np.asarray(keys, np.float32)[0].T)
    vT_full = np.ascontiguousarray(np.asarray(values, np.float32)[0].T)
    q32 = np.asarray(queries, np.float32)

    qT_n, kT_n, vT_n, vind_n = [], [], [], []
    for n in range(N):
        kt = np.zeros((KVDIM, KC), np.float32)
        vt = np.zeros((KVDIM, KC), np.float32)
        kt[:, :cnts[n]] = kT_full[:, idx[n]]
        vt[:, :cnts[n]] = vT_full[:, idx[n]]
        ind = (np.arange(KC) < cnts[n]).astype(np.float32)
        vind_n.append(np.ascontiguousarray(ind.reshape(T, 128).T).astype(BF16))
        kT_n.append(kt.astype(BF16))
        vT_n.append(vt.astype(BF16))
        qT_n.append(np.ascontiguousarray(q32[n].T).astype(BF16))
    return T, qT_n, kT_n, vT_n, vind_n


def kernel(queries, keys, values, mask, Wq, Wk, Wv, _trace=False):
    global last_exec_time_ns, last_results
    T, qT_n, kT_n, vT_n, vind_n = _prepare(queries, keys, values, mask)

    w_g = {}
    for nm, W in (("wq", Wq), ("wk", Wk), ("wv", Wv)):
        W = np.asarray(W, np.float32)
        w_g[nm] = [np.ascontiguousarray(W[:, g * 512:(g + 1) * 512]).astype(BF16)
                   for g in range(2)]

    nc = _cache.get(T)
    if nc is None:
        nc = _cache.setdefault(T, _build(T))

    in_maps = []
    for core in range(N_CORES):
        n, g = core // 2, core % 2
        in_maps.append({
            "qt": qT_n[n], "kt": kT_n[n], "vt": vT_n[n],
            "wq": w_g["wq"][g], "wk": w_g["wk"][g], "wv": w_g["wv"][g],
            "vind": vind_n[n],
        })

    res = run_bass_kernel_spmd(nc, in_maps, core_ids=list(range(N_CORES)),
                               trace=bool(_trace))
    last_exec_time_ns = res.exec_time_ns
    last_results = res

    full = np.empty((N, QLEN, EMBED), np.float32)
    for core in range(N_CORES):
        n, g = core // 2, core % 2
        av = np.asarray(res.results[core]["av"], dtype=np.float32)
        den = np.asarray(res.results[core]["den"], dtype=np.float32)
        av = av.reshape(4, 2, 64, QLEN)                  # [c, h, d, q]
        den = den.reshape(4, 128, QLEN)
        denom = np.stack([den[:, 0, :] + den[:, 64, :],
                          den[:, 32, :] + den[:, 96, :]], axis=1)  # [c, h, q]
        vals = av / denom[:, :, None, :]                 # [c, h, d, q]
        full[n, :, g * 512:(g + 1) * 512] = (
            vals.reshape(512, QLEN).T
        )
    return full


# revision 19
# speedup vs baseline: 1.1059x; 1.0391x over previous
"""MultiHeadCrossAttention Trainium2 kernel (8 NeuronCores, SPMD).

Sharding: core c -> (n = c // 2, g = c % 2). Each core handles one query
batch n and half the heads (8 of 16, embed slice g*512:(g+1)*512).

Host side: transpose queries/keys/values into [dim, tokens] layout, compact
keys/values along KLEN by the per-n mask (~50% survive), pad to KC = 128*T,
cast to bf16. The device returns unnormalized AV (bf16) plus per-head
softmax denominator partials; the host divides while assembling.

Device side per core (all matmuls bf16, fp32 PSUM accumulation), built
around PE array tiling (tile_position) so half-size matmuls run
concurrently in disjoint array quadrants:
  - energy: per (head-pair c, q-chunk, k-tile) ONE slot runs BOTH heads'
    K=64 energies concurrently as 2x row-tiled matmuls (rows 0:63 = head0,
    64:127 = head1, matching the natural qT/kT embed layout).
  - exp on ScalarE (scale=1/8): one ACTIVATE per k-tile covering both
    heads' [128, 512] PSUM banks via a strided [128, 2, 512] read.
  - AV: per k-tile ONE slot runs both heads as 2x col-tiled matmuls
    (M=64 each) accumulating into one PSUM bank (h0 -> partitions 0:64,
    h1 -> 64:128; single start/stop pair for the whole bank).
  - softmax denominators: 4x col-tiled M=1 matmuls (lhsT = validity
    indicator column) covering (2 heads x 2 k-tiles) per slot, landing at
    PSUM partitions {0,32,64,96} of a shared bank.
  - projections stream 512 cols per matmul (v-projection done once for
    all head-pairs in [token, emb] layout); proj work for pair c+1 is
    interleaved into pair c's attention stream to fill PE stalls while
    ScalarE (the bottleneck) streams exp continuously.
"""

import math
import sys
from collections import deque
from contextlib import ExitStack

import numpy as np

for _p in ("/opt/trn_rl_repo",):
    if _p not in sys.path:
        sys.path.insert(0, _p)

import ml_dtypes

import concourse.bass as bass  # noqa: F401  (import registers lowering deps)
import concourse.tile as tile
from concourse import bacc, mybir
from concourse.bass_utils import run_bass_kernel_spmd

BF16 = ml_dtypes.bfloat16

N, QLEN, KLEN = 4, 2048, 2048
QDIM = KVDIM = 512
EMBED, HEADS = 1024, 16
HEAD_DIM = 64
N_CORES = 8
QCH = 512  # q-chunk width (one PSUM bank of fp32)
SCALE = 1.0 / math.sqrt(HEAD_DIM)
# Schraudolph bf16 exp on the Vector engine: exp(x*SCALE) ~=
# bitcast_bf16(int16(round(x*A_SCH + B_SCH))); C=7.4 centers the ripple
# (zero-mean, ~1.8% rms; validated on HW). Applied to every 3rd k-tile to
# offload the exp-bound ScalarE; error contribution ~1.8%*sqrt(1/3) ~ 1%.
A_SCH = SCALE * 128.0 / math.log(2.0)
B_SCH = 127.0 * 128.0 - 7.4
DVE_EXP_PERIOD = 3  # every 3rd item's exp runs on DVE

_cache: dict = {}
last_exec_time_ns = None
last_results = None


def _build(T: int, ql: int = QLEN):
    """Build the per-core Bass program for KC = 128*T compacted kv tokens."""
    KC = 128 * T
    dt = mybir.dt
    nc = bacc.Bacc("TRN2", target_bir_lowering=False, debug=False)

    qT_d = nc.dram_tensor("qt", [QDIM, ql], dt.bfloat16, kind="ExternalInput").ap()
    kT_d = nc.dram_tensor("kt", [KVDIM, KC], dt.bfloat16, kind="ExternalInput").ap()
    vT_d = nc.dram_tensor("vt", [KVDIM, KC], dt.bfloat16, kind="ExternalInput").ap()
    wq_d = nc.dram_tensor("wq", [QDIM, 512], dt.bfloat16, kind="ExternalInput").ap()
    wk_d = nc.dram_tensor("wk", [KVDIM, 512], dt.bfloat16, kind="ExternalInput").ap()
    wv_d = nc.dram_tensor("wv", [KVDIM, 512], dt.bfloat16, kind="ExternalInput").ap()
    # per-row validity indicator (1.0 real kv token, 0.0 pad), [128, T]
    vind_d = nc.dram_tensor("vind", [128, T], dt.bfloat16, kind="ExternalInput").ap()
    # unnormalized AV.T: rows c*128 + h*64 + d, cols q
    av_d = nc.dram_tensor("av", [512, ql], dt.bfloat16, kind="ExternalOutput").ap()
    # denominator partials: rows c*128 + {0,32,64,96} = (h, k-tile parity)
    den_d = nc.dram_tensor("den", [512, ql], dt.bfloat16, kind="ExternalOutput").ap()

    NQ = ql // QCH
    kcols = [(s, min(512, KC - s)) for s in range(0, KC, 512)]

    with tile.TileContext(nc) as tc:
        with ExitStack() as ctx:
            persist = ctx.enter_context(tc.tile_pool(name="persist", bufs=1))

            qTin = [persist.tile([128, ql], dt.bfloat16, tag=f"qTin{j}", name=f"qTin{j}") for j in range(4)]
            kTin = [persist.tile([128, KC], dt.bfloat16, tag=f"kTin{j}", name=f"kTin{j}") for j in range(4)]
            vTin = [persist.tile([128, KC], dt.bfloat16, tag=f"vTin{j}", name=f"vTin{j}") for j in range(4)]
            wsb = {
                nm: [persist.tile([128, 512], dt.bfloat16, tag=f"{nm}{j}", name=f"{nm}{j}") for j in range(4)]
                for nm in ("wq", "wk", "wv")
            }
            qT = [persist.tile([128, ql], dt.bfloat16, tag=f"qT{c}", name=f"qT{c}") for c in range(4)]
            kT = [persist.tile([128, KC], dt.bfloat16, tag=f"kT{c}", name=f"kT{c}") for c in range(4)]
            # v in [token, emb-within-g] layout: AV lhsT for (c, h) is
            # vsb[:, t, c*128+h*64 : +64]
            vsb = persist.tile([128, T, 512], dt.bfloat16, tag="v", name="v")
            vind = persist.tile([128, T], dt.bfloat16, tag="vind", name="vind")
            junk = persist.tile([128, 512], dt.bfloat16, tag="junk", name="junk")

            # DMA order: k-side first so the first projections can start early
            for j in range(4):
                nc.sync.dma_start(wsb["wk"][j], wk_d[j * 128:(j + 1) * 128, :])
            for j in range(4):
                nc.sync.dma_start(kTin[j], kT_d[j * 128:(j + 1) * 128, :])
            for j in range(4):
                nc.sync.dma_start(wsb["wq"][j], wq_d[j * 128:(j + 1) * 128, :])
            for j in range(4):
                nc.sync.dma_start(qTin[j], qT_d[j * 128:(j + 1) * 128, :])
            for j in range(4):
                nc.sync.dma_start(wsb["wv"][j], wv_d[j * 128:(j + 1) * 128, :])
            for j in range(4):
                nc.sync.dma_start(vTin[j], vT_d[j * 128:(j + 1) * 128, :])
            nc.sync.dma_start(vind, vind_d)
            nc.vector.memset(junk, 1.0)

            with tc.tile_pool(name="psA", bufs=2, space="PSUM") as psA, \
                 tc.tile_pool(name="psE", bufs=2, space="PSUM") as psE, \
                 tc.tile_pool(name="psO", bufs=1, space="PSUM") as psO, \
                 tc.tile_pool(name="sbx", bufs=4) as sbx, \
                 tc.tile_pool(name="sbo", bufs=2) as sbo:

                # PE clock warm-up during the input-DMA window
                for _ in range(3):
                    ps = psA.tile([128, QCH], dt.float32, tag="pA", name="pA")
                    for r in range(10):
                        nc.tensor.matmul(ps, lhsT=junk[:, :128], rhs=junk,
                                         start=(r == 0), stop=(r == 9))

                # ---- projection emitters ----
                def emit_kproj(c, s, w):
                    ps = psA.tile([128, QCH], dt.float32, tag="pA", name="pA")
                    for j in range(4):
                        nc.tensor.matmul(
                            ps[:, :w],
                            lhsT=wsb["wk"][j][:, c * 128:(c + 1) * 128],
                            rhs=kTin[j][:, s:s + w],
                            start=(j == 0), stop=(j == 3),
                        )
                    nc.vector.tensor_copy(kT[c][:, s:s + w], ps[:, :w])

                def emit_qproj(c, q0):
                    ps = psA.tile([128, QCH], dt.float32, tag="pA", name="pA")
                    for j in range(4):
                        nc.tensor.matmul(
                            ps,
                            lhsT=wsb["wq"][j][:, c * 128:(c + 1) * 128],
                            rhs=qTin[j][:, q0 * QCH:(q0 + 1) * QCH],
                            start=(j == 0), stop=(j == 3),
                        )
                    nc.vector.tensor_copy(qT[c][:, q0 * QCH:(q0 + 1) * QCH], ps)

                def emit_vproj(t):
                    ps = psA.tile([128, QCH], dt.float32, tag="pA", name="pA")
                    for j in range(4):
                        nc.tensor.matmul(
                            ps,
                            lhsT=vTin[j][:, t * 128:(t + 1) * 128],
                            rhs=wsb["wv"][j],
                            start=(j == 0), stop=(j == 3),
                        )
                    nc.vector.tensor_copy(vsb[:, t, :], ps)

                # ---- attention emitters ----
                def emit_E(c, q0, t):
                    # both heads' K=64 energies concurrently (2x row tiling);
                    # flat [128, 1024] tile (contiguous 2 banks) so the exp
                    # ACTIVATE reads a single flat AP
                    eb = psE.tile([128, 2 * QCH], dt.float32, tag="e", name="e")
                    for h in range(2):
                        nc.tensor.matmul(
                            eb[:, h * QCH:(h + 1) * QCH],
                            lhsT=kT[c][h * 64:(h + 1) * 64, t * 128:(t + 1) * 128],
                            rhs=qT[c][h * 64:(h + 1) * 64, q0 * QCH:(q0 + 1) * QCH],
                            start=True, stop=True,
                        )
                    return eb

                def emit_X(eb, on_dve):
                    ex = sbx.tile([128, 2 * QCH], dt.bfloat16, tag="x", name="x")
                    if on_dve:
                        nc.vector.tensor_scalar(
                            ex.bitcast(dt.int16), eb, A_SCH, B_SCH,
                            mybir.AluOpType.mult, mybir.AluOpType.add)
                    else:
                        nc.scalar.activation(
                            ex, eb, mybir.ActivationFunctionType.Exp, scale=SCALE,
                        )
                    return ex

                def emit_A(c, t, ex, av):
                    # both heads' M=64 AV concurrently (2x col tiling);
                    # has_written is per-element on HW, so each col-tile
                    # region carries its own start/stop
                    for h in range(2):
                        nc.tensor.matmul(
                            av[h * 64:(h + 1) * 64, :],
                            lhsT=vsb[:, t, c * 128 + h * 64:c * 128 + (h + 1) * 64],
                            rhs=ex[:, h * QCH:(h + 1) * QCH],
                            start=(t == 0), stop=(t == T - 1),
                            skip_group_check=True,
                        )

                # last k-tile using each parity (for per-region stop flags)
                t_last = {0: ((T - 1) // 2) * 2, 1: ((T - 2) // 2) * 2 + 1 if T > 1 else None}

                def emit_D(pairs, den):
                    # 4x col-tiled M=1 denominator partials at partitions
                    # 64*(t%2) + 32*h of the shared bank; per-region
                    # start/stop (has_written is per-element on HW)
                    for (h, t, ex_t) in pairs:
                        pos = 64 * (t % 2) + 32 * h
                        nc.tensor.matmul(
                            den[pos:pos + 1, :],
                            lhsT=vind[:, t:t + 1],
                            rhs=ex_t[:, h * QCH:(h + 1) * QCH],
                            start=(t < 2), stop=(t == t_last[t % 2]),
                            skip_group_check=True,
                            tile_position=(0, pos),
                        )

                def emit_out(c, q0, av, den):
                    oav = sbo.tile([128, QCH], dt.bfloat16, tag="oav", name="oav")
                    nc.vector.tensor_copy(oav, av)
                    nc.sync.dma_start(
                        av_d[c * 128:(c + 1) * 128, q0 * QCH:(q0 + 1) * QCH], oav)
                    oden = sbo.tile([97, QCH], dt.bfloat16, tag="oden", name="oden")
                    nc.vector.tensor_copy(oden, den[0:97, :])
                    nc.sync.dma_start(
                        den_d[c * 128:c * 128 + 97, q0 * QCH:(q0 + 1) * QCH], oden)

                # ---- side-task queues: projections interleaved into the
                # attention stream (they fill PE stalls during exp waits) ----
                side = {c: deque() for c in range(4)}
                side[0].extend([(emit_vproj, (t,)) for t in range(2, T)])
                side[0].extend([(emit_qproj, (0, q0)) for q0 in (1, 2, 3)])
                for c in range(1, 4):
                    side[c].extend([(emit_kproj, (c, s, w)) for (s, w) in kcols])
                    side[c].extend([(emit_qproj, (c, q0)) for q0 in range(NQ)])

                # prefix: minimum projections for the first attention items
                for (s, w) in kcols:
                    emit_kproj(0, s, w)
                emit_qproj(0, 0)
                emit_vproj(0)
                if T > 1:
                    emit_vproj(1)

                # ---- software-pipelined attention stream ----
                stream = [(c, q0, t) for c in range(4) for q0 in range(NQ)
                          for t in range(T)]

                state = {}  # (c, q0) -> dict(av=, den=, ex={t: tile}, dfirst=)

                xctr = [0]

                def do_X(item, eb):
                    c, q0, t = item
                    st = state[(c, q0)]
                    xctr[0] += 1
                    ex = emit_X(eb, xctr[0] % DVE_EXP_PERIOD == 0)
                    st["ex"][t] = ex

                def do_A(item):
                    c, q0, t = item
                    st = state[(c, q0)]
                    emit_A(c, t, st["ex"][t], st["av"])
                    if t % 2 == 1 or t == T - 1:
                        pairs = []
                        for tt in ([t - 1, t] if t % 2 == 1 else [t]):
                            for h in range(2):
                                pairs.append((h, tt, st["ex"][tt]))
                        emit_D(pairs, st["den"])
                        for tt in set(tt for _, tt, _ in pairs):
                            st["ex"].pop(tt, None)
                    if t == T - 1:
                        emit_out(c, q0, st["av"], st["den"])
                        del state[(c, q0)]

                # software pipeline, per iteration j: E(j), X(j-1), A(j-2).
                # X(j-1) is emitted before E(j+1) (the next writer of its
                # psE buffer, bufs=2) so the WAR is tracked; every emitted
                # instruction's dependencies completed >= 1 full period ago,
                # so neither engine ever stalls mid-stream.
                pend_x = deque()
                pend_a = deque()
                for item in stream:
                    c, q0, t = item
                    if t == 0:
                        state[(c, q0)] = {
                            "av": psO.tile([128, QCH], dt.float32, tag="av", name="av"),
                            "den": psO.tile([128, QCH], dt.float32, tag="den", name="den"),
                            "ex": {},
                        }
                    eb = emit_E(c, q0, t)
                    # side projection task right after the energy pair: it
                    # fills the PE queue without gating anything downstream,
                    # and runs 2+ iterations before its outputs are consumed
                    sq = None
                    if side[c]:
                        sq = side[c]
                    elif c + 1 < 4 and side[c + 1]:
                        sq = side[c + 1]
                    if sq:
                        fn, args = sq.popleft()
                        fn(*args)
                    pend_x.append((item, eb))
                    if len(pend_x) >= 2:
                        it2, eb2 = pend_x.popleft()
                        do_X(it2, eb2)
                        pend_a.append(it2)
                    if len(pend_a) >= 2:
                        do_A(pend_a.popleft())
                while pend_x:
                    it2, eb2 = pend_x.popleft()
                    do_X(it2, eb2)
                    pend_a.append(it2)
                while pend_a:
                    do_A(pend_a.popleft())

    nc.compile()
    return nc


def _prepare(queries, keys, values, mask):
    """Host-side sharding: transpose, compact kv by mask, validity tiles."""
    m = np.asarray(mask).reshape(N, KLEN) != 0
    idx = [np.nonzero(m[n])[0] for n in range(N)]
    cnts = [len(i) for i in idx]
    T = max(1, (max(cnts) + 127) // 128)
    KC = 128 * T

    kT_full = np.ascontiguousarray(np.asarray(keys, np.float32)[0].T)
    vT_full = np.ascontiguousarray(np.asarray(values, np.float32)[0].T)
    q32 = np.asarray(queries, np.float32)

    qT_n, kT_n, vT_n, vind_n = [], [], [], []
    for n in range(N):
        kt = np.zeros((KVDIM, KC), np.float32)
        vt = np.zeros((KVDIM, KC), np.float32)
        kt[:, :cnts[n]] = kT_full[:, idx[n]]
        vt[:, :cnts[n]] = vT_full[:, idx[n]]
        ind = (np.arange(KC) < cnts[n]).astype(np.float32)
        vind_n.append(np.ascontiguousarray(ind.reshape(T, 128).T).astype(BF16))
        kT_n.append(kt.astype(BF16))
        vT_n.append(vt.astype(BF16))
        qT_n.append(np.ascontiguousarray(q32[n].T).astype(BF16))
    return T, qT_n, kT_n, vT_n, vind_n


def kernel(queries, keys, values, mask, Wq, Wk, Wv, _trace=False):
    global last_exec_time_ns, last_results
    T, qT_n, kT_n, vT_n, vind_n = _prepare(queries, keys, values, mask)

    w_g = {}
    for nm, W in (("wq", Wq), ("wk", Wk), ("wv", Wv)):
        W = np.asarray(W, np.float32)
        w_g[nm] = [np.ascontiguousarray(W[:, g * 512:(g + 1) * 512]).astype(BF16)
                   for g in range(2)]

    nc = _cache.get(T)
    if nc is None:
        nc = _cache.setdefault(T, _build(T))

    in_maps = []
    for core in range(N_CORES):
        n, g = core // 2, core % 2
        in_maps.append({
            "qt": qT_n[n], "kt": kT_n[n], "vt": vT_n[n],
            "wq": w_g["wq"][g], "wk": w_g["wk"][g], "wv": w_g["wv"][g],
            "vind": vind_n[n],
        })

    res = run_bass_kernel_spmd(nc, in_maps, core_ids=list(range(N_CORES)),
                               trace=bool(_trace))
    last_exec_time_ns = res.exec_time_ns
    last_results = res

    full = np.empty((N, QLEN, EMBED), np.float32)
    for core in range(N_CORES):
        n, g = core // 2, core % 2
        av = np.asarray(res.results[core]["av"], dtype=np.float32)
        den = np.asarray(res.results[core]["den"], dtype=np.float32)
        av = av.reshape(4, 2, 64, QLEN)                  # [c, h, d, q]
        den = den.reshape(4, 128, QLEN)
        denom = np.stack([den[:, 0, :] + den[:, 64, :],
                          den[:, 32, :] + den[:, 96, :]], axis=1)  # [c, h, q]
        vals = av / denom[:, :, None, :]                 # [c, h, d, q]
        full[n, :, g * 512:(g + 1) * 512] = (
            vals.reshape(512, QLEN).T
        )
    return full


# revision 25
# speedup vs baseline: 1.1351x; 1.0264x over previous
"""MultiHeadCrossAttention Trainium2 kernel (8 NeuronCores, SPMD).

Sharding: core c -> (n = c // 2, g = c % 2). Each core handles one query
batch n and half the heads (8 of 16, embed slice g*512:(g+1)*512).

Host side: transpose queries/keys/values into [dim, tokens] layout, compact
keys/values along KLEN by the per-n mask (~50% survive), pad to KC = 128*T,
cast to bf16. The device returns unnormalized AV (bf16) plus per-head
softmax denominator partials; the host divides while assembling.

Device side per core (all matmuls bf16, fp32 PSUM accumulation), built
around PE array tiling (tile_position) so half-size matmuls run
concurrently in disjoint array quadrants:
  - energy: per (head-pair c, q-chunk, k-tile) ONE slot runs BOTH heads'
    K=64 energies concurrently as 2x row-tiled matmuls (rows 0:63 = head0,
    64:127 = head1, matching the natural qT/kT embed layout).
  - exp on ScalarE (scale=1/8): one ACTIVATE per k-tile covering both
    heads' [128, 512] PSUM banks via a strided [128, 2, 512] read.
  - AV: per k-tile ONE slot runs both heads as 2x col-tiled matmuls
    (M=64 each) accumulating into one PSUM bank (h0 -> partitions 0:64,
    h1 -> 64:128; single start/stop pair for the whole bank).
  - softmax denominators: 4x col-tiled M=1 matmuls (lhsT = validity
    indicator column) covering (2 heads x 2 k-tiles) per slot, landing at
    PSUM partitions {0,32,64,96} of a shared bank.
  - projections stream 512 cols per matmul (v-projection done once for
    all head-pairs in [token, emb] layout); proj work for pair c+1 is
    interleaved into pair c's attention stream to fill PE stalls while
    ScalarE (the bottleneck) streams exp continuously.
"""

import math
import sys
from collections import deque
from contextlib import ExitStack

import numpy as np

for _p in ("/opt/trn_rl_repo",):
    if _p not in sys.path:
        sys.path.insert(0, _p)

import ml_dtypes

import concourse.bass as bass  # noqa: F401  (import registers lowering deps)
import concourse.tile as tile
from concourse import bacc, mybir
from concourse.bass_utils import run_bass_kernel_spmd

BF16 = ml_dtypes.bfloat16

N, QLEN, KLEN = 4, 2048, 2048
QDIM = KVDIM = 512
EMBED, HEADS = 1024, 16
HEAD_DIM = 64
N_CORES = 8
QCH = 512  # q-chunk width (one PSUM bank of fp32)
SCALE = 1.0 / math.sqrt(HEAD_DIM)
# Schraudolph bf16 exp on the Vector engine: exp(x*SCALE) ~=
# bitcast_bf16(int16(round(x*A_SCH + B_SCH))); C=7.4 centers the ripple
# (zero-mean, ~1.8% rms; validated on HW). Applied to every 3rd k-tile to
# offload the exp-bound ScalarE; error contribution ~1.8%*sqrt(1/3) ~ 1%.
A_SCH = SCALE * 128.0 / math.log(2.0)
B_SCH = 127.0 * 128.0 - 7.4
DVE_EXP_PERIOD = 4  # every 4th item's exp runs on DVE

_cache: dict = {}
last_exec_time_ns = None
last_results = None


def _build(T: int, ql: int = QLEN):
    """Build the per-core Bass program for KC = 128*T compacted kv tokens."""
    KC = 128 * T
    dt = mybir.dt
    nc = bacc.Bacc("TRN2", target_bir_lowering=False, debug=False)

    qT_d = nc.dram_tensor("qt", [QDIM, ql], dt.bfloat16, kind="ExternalInput").ap()
    kT_d = nc.dram_tensor("kt", [KVDIM, KC], dt.bfloat16, kind="ExternalInput").ap()
    vT_d = nc.dram_tensor("vt", [KVDIM, KC], dt.bfloat16, kind="ExternalInput").ap()
    wq_d = nc.dram_tensor("wq", [QDIM, 512], dt.bfloat16, kind="ExternalInput").ap()
    wk_d = nc.dram_tensor("wk", [KVDIM, 512], dt.bfloat16, kind="ExternalInput").ap()
    wv_d = nc.dram_tensor("wv", [KVDIM, 512], dt.bfloat16, kind="ExternalInput").ap()
    # validity indicator replicated 8x per local head: [128, T*8]
    # (1.0 real kv token, 0.0 pad)
    vind_d = nc.dram_tensor("vind", [128, T * 8], dt.bfloat16, kind="ExternalInput").ap()
    # unnormalized AV.T: rows (c*2+h)*65 + d (d=64 is the softmax
    # denominator), cols q
    av_d = nc.dram_tensor("av", [520, ql], dt.bfloat16, kind="ExternalOutput").ap()

    NQ = ql // QCH
    kcols = [(s, min(512, KC - s)) for s in range(0, KC, 512)]

    with tile.TileContext(nc) as tc:
        with ExitStack() as ctx:
            persist = ctx.enter_context(tc.tile_pool(name="persist", bufs=1))

            qTin = [persist.tile([128, ql], dt.bfloat16, tag=f"qTin{j}", name=f"qTin{j}") for j in range(4)]
            kTin = [persist.tile([128, KC], dt.bfloat16, tag=f"kTin{j}", name=f"kTin{j}") for j in range(4)]
            vTin = [persist.tile([128, KC], dt.bfloat16, tag=f"vTin{j}", name=f"vTin{j}") for j in range(4)]
            wsb = {
                nm: [persist.tile([128, 512], dt.bfloat16, tag=f"{nm}{j}", name=f"{nm}{j}") for j in range(4)]
                for nm in ("wq", "wk", "wv")
            }
            qT = [persist.tile([128, ql], dt.bfloat16, tag=f"qT{c}", name=f"qT{c}") for c in range(4)]
            kT = [persist.tile([128, KC], dt.bfloat16, tag=f"kT{c}", name=f"kT{c}") for c in range(4)]
            # v in [token, 8 x (64 v-dims | indicator)] layout: AV lhsT for
            # local head L = c*2+h is vsb[:, t, L*65 : L*65+65]; column
            # L*65+64 is the validity indicator, making row 64 of the AV
            # output the softmax denominator for free
            vsb = persist.tile([128, T, 520], dt.bfloat16, tag="v", name="v")
            vind = persist.tile([128, T, 8], dt.bfloat16, tag="vind", name="vind")
            junk = persist.tile([128, 512], dt.bfloat16, tag="junk", name="junk")

            # DMA order: k-side first so the first projections can start early
            for j in range(4):
                nc.sync.dma_start(wsb["wk"][j], wk_d[j * 128:(j + 1) * 128, :])
            for j in range(4):
                nc.sync.dma_start(kTin[j], kT_d[j * 128:(j + 1) * 128, :])
            for j in range(4):
                nc.sync.dma_start(wsb["wq"][j], wq_d[j * 128:(j + 1) * 128, :])
            for j in range(4):
                nc.sync.dma_start(qTin[j], qT_d[j * 128:(j + 1) * 128, :])
            for j in range(4):
                nc.sync.dma_start(wsb["wv"][j], wv_d[j * 128:(j + 1) * 128, :])
            for j in range(4):
                nc.sync.dma_start(vTin[j], vT_d[j * 128:(j + 1) * 128, :])
            nc.sync.dma_start(vind, vind_d)
            nc.vector.memset(junk, 1.0)

            with tc.tile_pool(name="psA", bufs=2, space="PSUM") as psA, \
                 tc.tile_pool(name="psE", bufs=2, space="PSUM") as psE, \
                 tc.tile_pool(name="psO", bufs=1, space="PSUM") as psO, \
                 tc.tile_pool(name="sbx", bufs=4) as sbx, \
                 tc.tile_pool(name="sbo", bufs=2) as sbo:

                # PE clock warm-up during the input-DMA window
                ps = psA.tile([128, QCH], dt.float32, tag="pA", name="pA")
                for r in range(12):
                    nc.tensor.matmul(ps, lhsT=junk[:, :128], rhs=junk,
                                     start=(r == 0), stop=(r == 11))
                # fill vsb's 8 indicator columns (stride-65 in the last dim)
                nc.vector.tensor_copy(vsb[:, :, 64::65], vind)

                # ---- projection emitters ----
                def emit_kproj(c, s, w):
                    ps = psA.tile([128, QCH], dt.float32, tag="pA", name="pA")
                    for j in range(4):
                        nc.tensor.matmul(
                            ps[:, :w],
                            lhsT=wsb["wk"][j][:, c * 128:(c + 1) * 128],
                            rhs=kTin[j][:, s:s + w],
                            start=(j == 0), stop=(j == 3),
                        )
                    nc.vector.tensor_copy(kT[c][:, s:s + w], ps[:, :w])

                def emit_qproj(c, q0):
                    ps = psA.tile([128, QCH], dt.float32, tag="pA", name="pA")
                    for j in range(4):
                        nc.tensor.matmul(
                            ps,
                            lhsT=wsb["wq"][j][:, c * 128:(c + 1) * 128],
                            rhs=qTin[j][:, q0 * QCH:(q0 + 1) * QCH],
                            start=(j == 0), stop=(j == 3),
                        )
                    nc.vector.tensor_copy(qT[c][:, q0 * QCH:(q0 + 1) * QCH], ps)

                def emit_vproj(t):
                    ps = psA.tile([128, QCH], dt.float32, tag="pA", name="pA")
                    for j in range(4):
                        nc.tensor.matmul(
                            ps,
                            lhsT=vTin[j][:, t * 128:(t + 1) * 128],
                            rhs=wsb["wv"][j],
                            start=(j == 0), stop=(j == 3),
                        )
                    for L in range(8):
                        nc.vector.tensor_copy(
                            vsb[:, t, L * 65:L * 65 + 64],
                            ps[:, L * 64:(L + 1) * 64])

                # ---- attention emitters ----
                def emit_E(c, q0, t):
                    # both heads' K=64 energies concurrently (2x row tiling);
                    # flat [128, 1024] tile (contiguous 2 banks) so the exp
                    # ACTIVATE reads a single flat AP
                    eb = psE.tile([128, 2 * QCH], dt.float32, tag="e", name="e")
                    for h in range(2):
                        nc.tensor.matmul(
                            eb[:, h * QCH:(h + 1) * QCH],
                            lhsT=kT[c][h * 64:(h + 1) * 64, t * 128:(t + 1) * 128],
                            rhs=qT[c][h * 64:(h + 1) * 64, q0 * QCH:(q0 + 1) * QCH],
                            start=True, stop=True,
                        )
                    return eb

                def emit_X(eb, on_dve):
                    ex = sbx.tile([128, 2 * QCH], dt.bfloat16, tag="x", name="x")
                    if on_dve:
                        nc.vector.tensor_scalar(
                            ex.bitcast(dt.int16), eb, A_SCH, B_SCH,
                            mybir.AluOpType.mult, mybir.AluOpType.add)
                    else:
                        nc.scalar.activation(
                            ex, eb, mybir.ActivationFunctionType.Exp, scale=SCALE,
                        )
                    return ex

                def emit_A(c, t, ex, av):
                    # per head: M=65 AV (64 v-dims + indicator column whose
                    # output row is the softmax denominator)
                    for h in range(2):
                        L = c * 2 + h
                        nc.tensor.matmul(
                            av[h][0:65, :],
                            lhsT=vsb[:, t, L * 65:L * 65 + 65],
                            rhs=ex[:, h * QCH:(h + 1) * QCH],
                            start=(t == 0), stop=(t == T - 1),
                        )

                def emit_out(c, q0, av):
                    for h in range(2):
                        L = c * 2 + h
                        oav = sbo.tile([65, QCH], dt.bfloat16, tag=f"oav{h}",
                                       name=f"oav{h}")
                        nc.vector.tensor_copy(oav, av[h][0:65, :])
                        nc.sync.dma_start(
                            av_d[L * 65:L * 65 + 65, q0 * QCH:(q0 + 1) * QCH],
                            oav)

                # ---- side-task queues: projections interleaved into the
                # attention stream (they fill PE stalls during exp waits) ----
                side = {c: deque() for c in range(4)}
                side[0].extend([(emit_vproj, (t,)) for t in range(2, T)])
                side[0].extend([(emit_qproj, (0, q0)) for q0 in (1, 2, 3)])
                for c in range(1, 4):
                    side[c].extend([(emit_kproj, (c, s, w)) for (s, w) in kcols])
                    side[c].extend([(emit_qproj, (c, q0)) for q0 in range(NQ)])

                # prefix: minimum projections for the first attention items
                for (s, w) in kcols:
                    emit_kproj(0, s, w)
                emit_qproj(0, 0)
                emit_vproj(0)
                if T > 1:
                    emit_vproj(1)

                # ---- software-pipelined attention stream ----
                stream = [(c, q0, t) for c in range(4) for q0 in range(NQ)
                          for t in range(T)]

                state = {}  # (c, q0) -> dict(av=, den=, ex={t: tile}, dfirst=)

                xctr = [0]

                def do_X(item, eb):
                    c, q0, t = item
                    st = state[(c, q0)]
                    xctr[0] += 1
                    ex = emit_X(eb, xctr[0] % DVE_EXP_PERIOD == 0)
                    st["ex"][t] = ex

                def do_A(item):
                    c, q0, t = item
                    st = state[(c, q0)]
                    emit_A(c, t, st["ex"][t], st["av"])
                    st["ex"].pop(t, None)
                    if t == T - 1:
                        emit_out(c, q0, st["av"])
                        del state[(c, q0)]

                # software pipeline, per iteration j: E(j), X(j-1), A(j-2).
                # X(j-1) is emitted before E(j+1) (the next writer of its
                # psE buffer, bufs=2) so the WAR is tracked; every emitted
                # instruction's dependencies completed >= 1 full period ago,
                # so neither engine ever stalls mid-stream.
                pend_x = deque()
                pend_a = deque()
                for item in stream:
                    c, q0, t = item
                    if t == 0:
                        state[(c, q0)] = {
                            "av": [psO.tile([128, QCH], dt.float32, tag=f"av{h}",
                                            name=f"av{h}") for h in range(2)],
                            "ex": {},
                        }
                    eb = emit_E(c, q0, t)
                    # side projection task right after the energy pair: it
                    # fills the PE queue without gating anything downstream,
                    # and runs 2+ iterations before its outputs are consumed
                    sq = None
                    if side[c]:
                        sq = side[c]
                    elif c + 1 < 4 and side[c + 1]:
                        sq = side[c + 1]
                    if sq:
                        fn, args = sq.popleft()
                        fn(*args)
                    pend_x.append((item, eb))
                    if len(pend_x) >= 2:
                        it2, eb2 = pend_x.popleft()
                        do_X(it2, eb2)
                        pend_a.append(it2)
                    if len(pend_a) >= 2:
                        do_A(pend_a.popleft())
                while pend_x:
                    it2, eb2 = pend_x.popleft()
                    do_X(it2, eb2)
                    pend_a.append(it2)
                while pend_a:
                    do_A(pend_a.popleft())

    nc.compile()
    return nc


def _prepare(queries, keys, values, mask):
    """Host-side sharding: transpose, compact kv by mask, validity tiles."""
    m = np.asarray(mask).reshape(N, KLEN) != 0
    idx = [np.nonzero(m[n])[0] for n in range(N)]
    cnts = [len(i) for i in idx]
    T = max(1, (max(cnts) + 127) // 128)
    KC = 128 * T

    kT_full = np.ascontiguousarray(np.asarray(keys, np.float32)[0].T)
    vT_full = np.ascontiguousarray(np.asarray(values, np.float32)[0].T)
    q32 = np.asarray(queries, np.float32)

    qT_n, kT_n, vT_n, vind_n = [], [], [], []
    for n in range(N):
        kt = np.zeros((KVDIM, KC), np.float32)
        vt = np.zeros((KVDIM, KC), np.float32)
        kt[:, :cnts[n]] = kT_full[:, idx[n]]
        vt[:, :cnts[n]] = vT_full[:, idx[n]]
        ind = (np.arange(KC) < cnts[n]).astype(np.float32)
        indT = ind.reshape(T, 128).T                       # [128, T]
        vind_n.append(np.ascontiguousarray(
            np.repeat(indT[:, :, None], 8, axis=2).reshape(128, T * 8)
        ).astype(BF16))
        kT_n.append(kt.astype(BF16))
        vT_n.append(vt.astype(BF16))
        qT_n.append(np.ascontiguousarray(q32[n].T).astype(BF16))
    return T, qT_n, kT_n, vT_n, vind_n


def kernel(queries, keys, values, mask, Wq, Wk, Wv, _trace=False):
    global last_exec_time_ns, last_results
    T, qT_n, kT_n, vT_n, vind_n = _prepare(queries, keys, values, mask)

    w_g = {}
    for nm, W in (("wq", Wq), ("wk", Wk), ("wv", Wv)):
        W = np.asarray(W, np.float32)
        w_g[nm] = [np.ascontiguousarray(W[:, g * 512:(g + 1) * 512]).astype(BF16)
                   for g in range(2)]

    nc = _cache.get(T)
    if nc is None:
        nc = _cache.setdefault(T, _build(T))

    in_maps = []
    for core in range(N_CORES):
        n, g = core // 2, core % 2
        in_maps.append({
            "qt": qT_n[n], "kt": kT_n[n], "vt": vT_n[n],
            "wq": w_g["wq"][g], "wk": w_g["wk"][g], "wv": w_g["wv"][g],
            "vind": vind_n[n],
        })

    res = run_bass_kernel_spmd(nc, in_maps, core_ids=list(range(N_CORES)),
                               trace=bool(_trace))
    last_exec_time_ns = res.exec_time_ns
    last_results = res

    full = np.empty((N, QLEN, EMBED), np.float32)
    for core in range(N_CORES):
        n, g = core // 2, core % 2
        o = np.asarray(res.results[core]["av"], dtype=np.float32)
        o = o.reshape(8, 65, QLEN)                       # [L, d|denom, q]
        vals = o[:, :64, :] / o[:, 64:65, :]             # [8, 64, QLEN]
        full[n, :, g * 512:(g + 1) * 512] = (
            vals.transpose(2, 0, 1).reshape(QLEN, 512)
        )
    return full


# revision 26
# speedup vs baseline: 1.1395x; 1.0039x over previous
"""MultiHeadCrossAttention Trainium2 kernel (8 NeuronCores, SPMD).

Sharding: core c -> (n = c // 2, g = c % 2). Each core handles one query
batch n and half the heads (8 of 16, embed slice g*512:(g+1)*512).

Host side: transpose queries/keys/values into [dim, tokens] layout, compact
keys/values along KLEN by the per-n mask (~50% survive), pad to KC = 128*T,
cast to bf16. The device returns unnormalized AV (bf16) plus per-head
softmax denominator partials; the host divides while assembling.

Device side per core (all matmuls bf16, fp32 PSUM accumulation), built
around PE array tiling (tile_position) so half-size matmuls run
concurrently in disjoint array quadrants:
  - energy: per (head-pair c, q-chunk, k-tile) ONE slot runs BOTH heads'
    K=64 energies concurrently as 2x row-tiled matmuls (rows 0:63 = head0,
    64:127 = head1, matching the natural qT/kT embed layout).
  - exp on ScalarE (scale=1/8): one ACTIVATE per k-tile covering both
    heads' [128, 512] PSUM banks via a strided [128, 2, 512] read.
  - AV: per k-tile ONE slot runs both heads as 2x col-tiled matmuls
    (M=64 each) accumulating into one PSUM bank (h0 -> partitions 0:64,
    h1 -> 64:128; single start/stop pair for the whole bank).
  - softmax denominators: 4x col-tiled M=1 matmuls (lhsT = validity
    indicator column) covering (2 heads x 2 k-tiles) per slot, landing at
    PSUM partitions {0,32,64,96} of a shared bank.
  - projections stream 512 cols per matmul (v-projection done once for
    all head-pairs in [token, emb] layout); proj work for pair c+1 is
    interleaved into pair c's attention stream to fill PE stalls while
    ScalarE (the bottleneck) streams exp continuously.
"""

import math
import sys
from collections import deque
from contextlib import ExitStack

import numpy as np

for _p in ("/opt/trn_rl_repo",):
    if _p not in sys.path:
        sys.path.insert(0, _p)

import ml_dtypes

import concourse.bass as bass  # noqa: F401  (import registers lowering deps)
import concourse.tile as tile
from concourse import bacc, mybir
from concourse.bass_utils import run_bass_kernel_spmd

BF16 = ml_dtypes.bfloat16

N, QLEN, KLEN = 4, 2048, 2048
QDIM = KVDIM = 512
EMBED, HEADS = 1024, 16
HEAD_DIM = 64
N_CORES = 8
QCH = 512  # q-chunk width (one PSUM bank of fp32)
SCALE = 1.0 / math.sqrt(HEAD_DIM)
# Schraudolph bf16 exp on the Vector engine: exp(x*SCALE) ~=
# bitcast_bf16(int16(round(x*A_SCH + B_SCH))); C=7.4 centers the ripple
# (zero-mean, ~1.8% rms; validated on HW). Applied to every 3rd k-tile to
# offload the exp-bound ScalarE; error contribution ~1.8%*sqrt(1/3) ~ 1%.
A_SCH = SCALE * 128.0 / math.log(2.0)
B_SCH = 127.0 * 128.0 - 7.4
DVE_EXP_PERIOD = 4  # every 4th item's exp runs on DVE

_cache: dict = {}
last_exec_time_ns = None
last_results = None


def _build(T: int, ql: int = QLEN):
    """Build the per-core Bass program for KC = 128*T compacted kv tokens."""
    KC = 128 * T
    dt = mybir.dt
    nc = bacc.Bacc("TRN2", target_bir_lowering=False, debug=False)

    qT_d = nc.dram_tensor("qt", [QDIM, ql], dt.bfloat16, kind="ExternalInput").ap()
    kT_d = nc.dram_tensor("kt", [KVDIM, KC], dt.bfloat16, kind="ExternalInput").ap()
    vT_d = nc.dram_tensor("vt", [KVDIM, KC], dt.bfloat16, kind="ExternalInput").ap()
    wq_d = nc.dram_tensor("wq", [QDIM, 512], dt.bfloat16, kind="ExternalInput").ap()
    wk_d = nc.dram_tensor("wk", [KVDIM, 512], dt.bfloat16, kind="ExternalInput").ap()
    wv_d = nc.dram_tensor("wv", [KVDIM, 512], dt.bfloat16, kind="ExternalInput").ap()
    # validity indicator replicated 8x per local head: [128, T*8]
    # (1.0 real kv token, 0.0 pad)
    vind_d = nc.dram_tensor("vind", [128, T * 8], dt.bfloat16, kind="ExternalInput").ap()
    # unnormalized AV.T: rows (c*2+h)*65 + d (d=64 is the softmax
    # denominator), cols q
    av_d = nc.dram_tensor("av", [520, ql], dt.bfloat16, kind="ExternalOutput").ap()

    NQ = ql // QCH
    kcols = [(s, min(512, KC - s)) for s in range(0, KC, 512)]

    with tile.TileContext(nc) as tc:
        with ExitStack() as ctx:
            persist = ctx.enter_context(tc.tile_pool(name="persist", bufs=1))

            qTin = [persist.tile([128, ql], dt.bfloat16, tag=f"qTin{j}", name=f"qTin{j}") for j in range(4)]
            kTin = [persist.tile([128, KC], dt.bfloat16, tag=f"kTin{j}", name=f"kTin{j}") for j in range(4)]
            vTin = [persist.tile([128, KC], dt.bfloat16, tag=f"vTin{j}", name=f"vTin{j}") for j in range(4)]
            wsb = {
                nm: [persist.tile([128, 512], dt.bfloat16, tag=f"{nm}{j}", name=f"{nm}{j}") for j in range(4)]
                for nm in ("wq", "wk", "wv")
            }
            qT = [persist.tile([128, ql], dt.bfloat16, tag=f"qT{c}", name=f"qT{c}") for c in range(4)]
            kT = [persist.tile([128, KC], dt.bfloat16, tag=f"kT{c}", name=f"kT{c}") for c in range(4)]
            # v in [token, 8 x (64 v-dims | indicator)] layout: AV lhsT for
            # local head L = c*2+h is vsb[:, t, L*65 : L*65+65]; column
            # L*65+64 is the validity indicator, making row 64 of the AV
            # output the softmax denominator for free
            vsb = persist.tile([128, T, 8, 65], dt.bfloat16, tag="v", name="v")
            junk = persist.tile([128, 512], dt.bfloat16, tag="junk", name="junk")

            # DMA order: k-side first so the first projections can start early
            nc.sync.dma_start(vsb[:, :, :, 64], vind_d)
            for j in range(4):
                nc.sync.dma_start(wsb["wk"][j], wk_d[j * 128:(j + 1) * 128, :])
            for j in range(4):
                nc.sync.dma_start(kTin[j], kT_d[j * 128:(j + 1) * 128, :])
            for j in range(4):
                nc.sync.dma_start(wsb["wq"][j], wq_d[j * 128:(j + 1) * 128, :])
            for j in range(4):
                nc.sync.dma_start(qTin[j], qT_d[j * 128:(j + 1) * 128, :])
            for j in range(4):
                nc.sync.dma_start(wsb["wv"][j], wv_d[j * 128:(j + 1) * 128, :])
            for j in range(4):
                nc.sync.dma_start(vTin[j], vT_d[j * 128:(j + 1) * 128, :])
            nc.vector.memset(junk, 1.0)

            with tc.tile_pool(name="psA", bufs=2, space="PSUM") as psA, \
                 tc.tile_pool(name="psE", bufs=2, space="PSUM") as psE, \
                 tc.tile_pool(name="psO", bufs=1, space="PSUM") as psO, \
                 tc.tile_pool(name="sbx", bufs=4) as sbx, \
                 tc.tile_pool(name="sbo", bufs=2) as sbo:

                # PE clock warm-up during the input-DMA window
                ps = psA.tile([128, QCH], dt.float32, tag="pA", name="pA")
                for r in range(12):
                    nc.tensor.matmul(ps, lhsT=junk[:, :128], rhs=junk,
                                     start=(r == 0), stop=(r == 11))

                # ---- projection emitters ----
                def emit_kproj(c, s, w):
                    ps = psA.tile([128, QCH], dt.float32, tag="pA", name="pA")
                    for j in range(4):
                        nc.tensor.matmul(
                            ps[:, :w],
                            lhsT=wsb["wk"][j][:, c * 128:(c + 1) * 128],
                            rhs=kTin[j][:, s:s + w],
                            start=(j == 0), stop=(j == 3),
                        )
                    nc.vector.tensor_copy(kT[c][:, s:s + w], ps[:, :w])

                def emit_qproj(c, q0):
                    ps = psA.tile([128, QCH], dt.float32, tag="pA", name="pA")
                    for j in range(4):
                        nc.tensor.matmul(
                            ps,
                            lhsT=wsb["wq"][j][:, c * 128:(c + 1) * 128],
                            rhs=qTin[j][:, q0 * QCH:(q0 + 1) * QCH],
                            start=(j == 0), stop=(j == 3),
                        )
                    nc.vector.tensor_copy(qT[c][:, q0 * QCH:(q0 + 1) * QCH], ps)

                def emit_vproj(t):
                    ps = psA.tile([128, 8, 64], dt.float32, tag="pA", name="pA")
                    for j in range(4):
                        nc.tensor.matmul(
                            ps,
                            lhsT=vTin[j][:, t * 128:(t + 1) * 128],
                            rhs=wsb["wv"][j],
                            start=(j == 0), stop=(j == 3),
                        )
                    nc.vector.tensor_copy(vsb[:, t, :, 0:64], ps)

                # ---- attention emitters ----
                def emit_E(c, q0, t):
                    # both heads' K=64 energies concurrently (2x row tiling);
                    # flat [128, 1024] tile (contiguous 2 banks) so the exp
                    # ACTIVATE reads a single flat AP
                    eb = psE.tile([128, 2 * QCH], dt.float32, tag="e", name="e")
                    for h in range(2):
                        nc.tensor.matmul(
                            eb[:, h * QCH:(h + 1) * QCH],
                            lhsT=kT[c][h * 64:(h + 1) * 64, t * 128:(t + 1) * 128],
                            rhs=qT[c][h * 64:(h + 1) * 64, q0 * QCH:(q0 + 1) * QCH],
                            start=True, stop=True,
                        )
                    return eb

                def emit_X(eb, on_dve):
                    ex = sbx.tile([128, 2 * QCH], dt.bfloat16,
                                  tag="xv" if on_dve else "xs", name="x")
                    if on_dve:
                        nc.vector.tensor_scalar(
                            ex.bitcast(dt.int16), eb, A_SCH, B_SCH,
                            mybir.AluOpType.mult, mybir.AluOpType.add)
                    else:
                        nc.scalar.activation(
                            ex, eb, mybir.ActivationFunctionType.Exp, scale=SCALE,
                        )
                    return ex

                def emit_A(c, t, ex, av):
                    # per head: M=65 AV (64 v-dims + indicator column whose
                    # output row is the softmax denominator)
                    for h in range(2):
                        L = c * 2 + h
                        nc.tensor.matmul(
                            av[h][0:65, :],
                            lhsT=vsb[:, t, L, :],
                            rhs=ex[:, h * QCH:(h + 1) * QCH],
                            start=(t == 0), stop=(t == T - 1),
                        )

                def emit_out(c, q0, av):
                    for h in range(2):
                        L = c * 2 + h
                        oav = sbo.tile([65, QCH], dt.bfloat16, tag=f"oav{h}",
                                       name=f"oav{h}")
                        nc.vector.tensor_copy(oav, av[h][0:65, :])
                        nc.sync.dma_start(
                            av_d[L * 65:L * 65 + 65, q0 * QCH:(q0 + 1) * QCH],
                            oav)

                # ---- side-task queues: projections interleaved into the
                # attention stream (they fill PE stalls during exp waits) ----
                side = {c: deque() for c in range(4)}
                side[0].extend([(emit_vproj, (t,)) for t in range(2, T)])
                side[0].extend([(emit_qproj, (0, q0)) for q0 in (1, 2, 3)])
                for c in range(1, 4):
                    side[c].extend([(emit_kproj, (c, s, w)) for (s, w) in kcols])
                    side[c].extend([(emit_qproj, (c, q0)) for q0 in range(NQ)])

                # prefix: minimum projections for the first attention items
                for (s, w) in kcols:
                    emit_kproj(0, s, w)
                emit_qproj(0, 0)
                emit_vproj(0)
                if T > 1:
                    emit_vproj(1)

                # ---- software-pipelined attention stream ----
                stream = [(c, q0, t) for c in range(4) for q0 in range(NQ)
                          for t in range(T)]

                state = {}  # (c, q0) -> dict(av=, den=, ex={t: tile}, dfirst=)

                xctr = [0]

                def do_X(item, eb):
                    c, q0, t = item
                    st = state[(c, q0)]
                    xctr[0] += 1
                    on_dve = xctr[0] > 16 and xctr[0] % DVE_EXP_PERIOD == 0
                    ex = emit_X(eb, on_dve)
                    st["ex"][t] = ex

                def do_A(item):
                    c, q0, t = item
                    st = state[(c, q0)]
                    emit_A(c, t, st["ex"][t], st["av"])
                    st["ex"].pop(t, None)
                    if t == T - 1:
                        emit_out(c, q0, st["av"])
                        del state[(c, q0)]

                # software pipeline, per iteration j: E(j), X(j-1), A(j-2).
                # X(j-1) is emitted before E(j+1) (the next writer of its
                # psE buffer, bufs=2) so the WAR is tracked; every emitted
                # instruction's dependencies completed >= 1 full period ago,
                # so neither engine ever stalls mid-stream.
                pend_x = deque()
                pend_a = deque()
                for item in stream:
                    c, q0, t = item
                    if t == 0:
                        state[(c, q0)] = {
                            "av": [psO.tile([128, QCH], dt.float32, tag=f"av{h}",
                                            name=f"av{h}") for h in range(2)],
                            "ex": {},
                        }
                    eb = emit_E(c, q0, t)
                    # side projection task right after the energy pair: it
                    # fills the PE queue without gating anything downstream,
                    # and runs 2+ iterations before its outputs are consumed
                    sq = None
                    if side[c]:
                        sq = side[c]
                    elif c + 1 < 4 and side[c + 1]:
                        sq = side[c + 1]
                    if sq:
                        fn, args = sq.popleft()
                        fn(*args)
                    pend_x.append((item, eb))
                    if len(pend_x) >= 2:
                        it2, eb2 = pend_x.popleft()
                        do_X(it2, eb2)
                        pend_a.append(it2)
                    if len(pend_a) >= 2:
                        do_A(pend_a.popleft())
                while pend_x:
                    it2, eb2 = pend_x.popleft()
                    do_X(it2, eb2)
                    pend_a.append(it2)
                while pend_a:
                    do_A(pend_a.popleft())

    nc.compile()
    return nc


def _prepare(queries, keys, values, mask):
    """Host-side sharding: transpose, compact kv by mask, validity tiles."""
    m = np.asarray(mask).reshape(N, KLEN) != 0
    idx = [np.nonzero(m[n])[0] for n in range(N)]
    cnts = [len(i) for i in idx]
    T = max(1, (max(cnts) + 127) // 128)
    KC = 128 * T

    kT_full = np.ascontiguousarray(np.asarray(keys, np.float32)[0].T)
    vT_full = np.ascontiguousarray(np.asarray(values, np.float32)[0].T)
    q32 = np.asarray(queries, np.float32)

    qT_n, kT_n, vT_n, vind_n = [], [], [], []
    for n in range(N):
        kt = np.zeros((KVDIM, KC), np.float32)
        vt = np.zeros((KVDIM, KC), np.float32)
        kt[:, :cnts[n]] = kT_full[:, idx[n]]
        vt[:, :cnts[n]] = vT_full[:, idx[n]]
        ind = (np.arange(KC) < cnts[n]).astype(np.float32)
        indT = ind.reshape(T, 128).T                       # [128, T]
        vind_n.append(np.ascontiguousarray(
            np.repeat(indT[:, :, None], 8, axis=2).reshape(128, T * 8)
        ).astype(BF16))
        kT_n.append(kt.astype(BF16))
        vT_n.append(vt.astype(BF16))
        qT_n.append(np.ascontiguousarray(q32[n].T).astype(BF16))
    return T, qT_n, kT_n, vT_n, vind_n


def kernel(queries, keys, values, mask, Wq, Wk, Wv, _trace=False):
    global last_exec_time_ns, last_results
    T, qT_n, kT_n, vT_n, vind_n = _prepare(queries, keys, values, mask)

    w_g = {}
    for nm, W in (("wq", Wq), ("wk", Wk), ("wv", Wv)):
        W = np.asarray(W, np.float32)
        w_g[nm] = [np.ascontiguousarray(W[:, g * 512:(g + 1) * 512]).astype(BF16)
                   for g in range(2)]

    nc = _cache.get(T)
    if nc is None:
        nc = _cache.setdefault(T, _build(T))

    in_maps = []
    for core in range(N_CORES):
        n, g = core // 2, core % 2
        in_maps.append({
            "qt": qT_n[n], "kt": kT_n[n], "vt": vT_n[n],
            "wq": w_g["wq"][g], "wk": w_g["wk"][g], "wv": w_g["wv"][g],
            "vind": vind_n[n],
        })

    res = run_bass_kernel_spmd(nc, in_maps, core_ids=list(range(N_CORES)),
                               trace=bool(_trace))
    last_exec_time_ns = res.exec_time_ns
    last_results = res

    full = np.empty((N, QLEN, EMBED), np.float32)
    for core in range(N_CORES):
        n, g = core // 2, core % 2
        o = np.asarray(res.results[core]["av"], dtype=np.float32)
        o = o.reshape(8, 65, QLEN)                       # [L, d|denom, q]
        vals = o[:, :64, :] / o[:, 64:65, :]             # [8, 64, QLEN]
        full[n, :, g * 512:(g + 1) * 512] = (
            vals.transpose(2, 0, 1).reshape(QLEN, 512)
        )
    return full


# revision 29
# speedup vs baseline: 1.1580x; 1.0162x over previous
"""MultiHeadCrossAttention Trainium2 kernel (8 NeuronCores, SPMD).

Sharding: core c -> (n = c // 2, g = c % 2). Each core handles one query
batch n and half the heads (8 of 16, embed slice g*512:(g+1)*512).

Host side: transpose queries/keys/values into [dim, tokens] layout, compact
keys/values along KLEN by the per-n mask (~50% survive), pad to KC = 128*T,
cast to bf16. The device returns unnormalized AV (bf16) plus per-head
softmax denominator partials; the host divides while assembling.

Device side per core (all matmuls bf16, fp32 PSUM accumulation), built
around PE array tiling (tile_position) so half-size matmuls run
concurrently in disjoint array quadrants:
  - energy: per (head-pair c, q-chunk, k-tile) ONE slot runs BOTH heads'
    K=64 energies concurrently as 2x row-tiled matmuls (rows 0:63 = head0,
    64:127 = head1, matching the natural qT/kT embed layout).
  - exp on ScalarE (scale=1/8): one ACTIVATE per k-tile covering both
    heads' [128, 512] PSUM banks via a strided [128, 2, 512] read.
  - AV: per k-tile ONE slot runs both heads as 2x col-tiled matmuls
    (M=64 each) accumulating into one PSUM bank (h0 -> partitions 0:64,
    h1 -> 64:128; single start/stop pair for the whole bank).
  - softmax denominators: 4x col-tiled M=1 matmuls (lhsT = validity
    indicator column) covering (2 heads x 2 k-tiles) per slot, landing at
    PSUM partitions {0,32,64,96} of a shared bank.
  - projections stream 512 cols per matmul (v-projection done once for
    all head-pairs in [token, emb] layout); proj work for pair c+1 is
    interleaved into pair c's attention stream to fill PE stalls while
    ScalarE (the bottleneck) streams exp continuously.
"""

import math
import sys
from collections import deque
from contextlib import ExitStack

import numpy as np

for _p in ("/opt/trn_rl_repo",):
    if _p not in sys.path:
        sys.path.insert(0, _p)

import ml_dtypes

import concourse.bass as bass  # noqa: F401  (import registers lowering deps)
import concourse.tile as tile
from concourse import bacc, mybir
from concourse.bass_utils import run_bass_kernel_spmd

BF16 = ml_dtypes.bfloat16

N, QLEN, KLEN = 4, 2048, 2048
QDIM = KVDIM = 512
EMBED, HEADS = 1024, 16
HEAD_DIM = 64
N_CORES = 8
QCH = 512  # q-chunk width (one PSUM bank of fp32)
SCALE = 1.0 / math.sqrt(HEAD_DIM)
# Schraudolph bf16 exp on the Vector engine: exp(x*SCALE) ~=
# bitcast_bf16(int16(round(x*A_SCH + B_SCH))); C=7.4 centers the ripple
# (zero-mean, ~1.8% rms; validated on HW). Applied to every 3rd k-tile to
# offload the exp-bound ScalarE; error contribution ~1.8%*sqrt(1/3) ~ 1%.
A_SCH = SCALE * 128.0 / math.log(2.0)
B_SCH = 127.0 * 128.0 - 7.4
DVE_EXP_PERIOD = 4  # every 4th item's exp runs on DVE

_cache: dict = {}
last_exec_time_ns = None
last_results = None


def _build(T: int, ql: int = QLEN):
    """Build the per-core Bass program for KC = 128*T compacted kv tokens."""
    KC = 128 * T
    dt = mybir.dt
    nc = bacc.Bacc("TRN2", target_bir_lowering=False, debug=False)

    # host pre-interleaves the 128-partition-major layout: x[p, j, c]
    qT_d = nc.dram_tensor("qt", [128, 4, ql], dt.bfloat16, kind="ExternalInput").ap()
    kT_d = nc.dram_tensor("kt", [128, 4, KC], dt.bfloat16, kind="ExternalInput").ap()
    vT_d = nc.dram_tensor("vt", [128, 4, KC], dt.bfloat16, kind="ExternalInput").ap()
    wq_d = nc.dram_tensor("wq", [128, 4, 512], dt.bfloat16, kind="ExternalInput").ap()
    wk_d = nc.dram_tensor("wk", [128, 4, 512], dt.bfloat16, kind="ExternalInput").ap()
    wv_d = nc.dram_tensor("wv", [128, 4, 512], dt.bfloat16, kind="ExternalInput").ap()
    # validity indicator replicated 8x per local head: [128, T*8]
    # (1.0 real kv token, 0.0 pad)
    vind_d = nc.dram_tensor("vind", [128, T * 8], dt.bfloat16, kind="ExternalInput").ap()
    # unnormalized AV.T: rows (c*2+h)*65 + d (d=64 is the softmax
    # denominator), cols q
    av_d = nc.dram_tensor("av", [520, ql], dt.bfloat16, kind="ExternalOutput").ap()

    NQ = ql // QCH
    kcols = [(s, min(512, KC - s)) for s in range(0, KC, 512)]

    with tile.TileContext(nc) as tc:
        with ExitStack() as ctx:
            persist = ctx.enter_context(tc.tile_pool(name="persist", bufs=1))

            qTin_t = persist.tile([128, 4, ql], dt.bfloat16, tag="qTin", name="qTin")
            kTin_t = persist.tile([128, 4, KC], dt.bfloat16, tag="kTin", name="kTin")
            vTin_t = persist.tile([128, 4, KC], dt.bfloat16, tag="vTin", name="vTin")
            qTin = [qTin_t[:, j, :] for j in range(4)]
            kTin = [kTin_t[:, j, :] for j in range(4)]
            vTin = [vTin_t[:, j, :] for j in range(4)]
            wsb_t = {nm: persist.tile([128, 4, 512], dt.bfloat16, tag=nm, name=nm)
                     for nm in ("wq", "wk", "wv")}
            wsb = {nm: [wsb_t[nm][:, j, :] for j in range(4)]
                   for nm in ("wq", "wk", "wv")}
            qT = [persist.tile([128, ql], dt.bfloat16, tag=f"qT{c}", name=f"qT{c}") for c in range(4)]
            kT = [persist.tile([128, KC], dt.bfloat16, tag=f"kT{c}", name=f"kT{c}") for c in range(4)]
            # v in [token, 8 x (64 v-dims | indicator)] layout: AV lhsT for
            # local head L = c*2+h is vsb[:, t, L*65 : L*65+65]; column
            # L*65+64 is the validity indicator, making row 64 of the AV
            # output the softmax denominator for free
            vsb = persist.tile([128, T, 8, 65], dt.bfloat16, tag="v", name="v")
            junk = persist.tile([128, 512], dt.bfloat16, tag="junk", name="junk")

            vindst = persist.tile([128, T, 8], dt.bfloat16, tag="vi", name="vi")
            # DMA order: k-side first so the first projections can start early
            nc.sync.dma_start(wsb_t["wk"], wk_d)
            nc.sync.dma_start(kTin_t, kT_d)
            nc.sync.dma_start(wsb_t["wq"], wq_d)
            nc.sync.dma_start(qTin_t, qT_d)
            nc.sync.dma_start(wsb_t["wv"], wv_d)
            nc.sync.dma_start(vTin_t, vT_d)
            nc.sync.dma_start(vindst, vind_d)
            nc.vector.memset(junk, 1.0)

            with tc.tile_pool(name="psA", bufs=2, space="PSUM") as psA, \
                 tc.tile_pool(name="psE", bufs=2, space="PSUM") as psE, \
                 tc.tile_pool(name="psO", bufs=1, space="PSUM") as psO, \
                 tc.tile_pool(name="sbx", bufs=4) as sbx, \
                 tc.tile_pool(name="sbo", bufs=2) as sbo:

                # PE clock warm-up during the input-DMA window
                ps = psA.tile([128, QCH], dt.float32, tag="pA", name="pA")
                for r in range(12):
                    nc.tensor.matmul(ps, lhsT=junk[:, :128], rhs=junk,
                                     start=(r == 0), stop=(r == 11))

                # ---- projection emitters ----
                def emit_kproj(c, s, w):
                    ps = psA.tile([128, QCH], dt.float32, tag="pA", name="pA")
                    for j in range(4):
                        nc.tensor.matmul(
                            ps[:, :w],
                            lhsT=wsb["wk"][j][:, c * 128:(c + 1) * 128],
                            rhs=kTin[j][:, s:s + w],
                            start=(j == 0), stop=(j == 3),
                        )
                    nc.vector.tensor_copy(kT[c][:, s:s + w], ps[:, :w])

                def emit_qproj(c, q0):
                    ps = psA.tile([128, QCH], dt.float32, tag="pA", name="pA")
                    for j in range(4):
                        nc.tensor.matmul(
                            ps,
                            lhsT=wsb["wq"][j][:, c * 128:(c + 1) * 128],
                            rhs=qTin[j][:, q0 * QCH:(q0 + 1) * QCH],
                            start=(j == 0), stop=(j == 3),
                        )
                    nc.vector.tensor_copy(qT[c][:, q0 * QCH:(q0 + 1) * QCH], ps)

                def emit_vproj(t):
                    ps = psA.tile([128, 8, 64], dt.float32, tag="pA", name="pA")
                    for j in range(4):
                        nc.tensor.matmul(
                            ps,
                            lhsT=vTin[j][:, t * 128:(t + 1) * 128],
                            rhs=wsb["wv"][j],
                            start=(j == 0), stop=(j == 3),
                        )
                    nc.vector.tensor_copy(vsb[:, t, :, 0:64], ps)

                # ---- attention emitters ----
                def emit_E(c, q0, t):
                    # both heads' K=64 energies concurrently (2x row tiling);
                    # flat [128, 1024] tile (contiguous 2 banks) so the exp
                    # ACTIVATE reads a single flat AP
                    eb = psE.tile([128, 2 * QCH], dt.float32, tag="e", name="e")
                    for h in range(2):
                        nc.tensor.matmul(
                            eb[:, h * QCH:(h + 1) * QCH],
                            lhsT=kT[c][h * 64:(h + 1) * 64, t * 128:(t + 1) * 128],
                            rhs=qT[c][h * 64:(h + 1) * 64, q0 * QCH:(q0 + 1) * QCH],
                            start=True, stop=True,
                        )
                    return eb

                def emit_X(eb, on_dve):
                    ex = sbx.tile([128, 2 * QCH], dt.bfloat16,
                                  tag="xv" if on_dve else "xs", name="x")
                    if on_dve:
                        nc.vector.tensor_scalar(
                            ex.bitcast(dt.int16), eb, A_SCH, B_SCH,
                            mybir.AluOpType.mult, mybir.AluOpType.add)
                    else:
                        nc.scalar.activation(
                            ex, eb, mybir.ActivationFunctionType.Exp, scale=SCALE,
                        )
                    return ex

                def emit_A(c, t, ex, av):
                    # per head: M=65 AV (64 v-dims + indicator column whose
                    # output row is the softmax denominator)
                    for h in range(2):
                        L = c * 2 + h
                        nc.tensor.matmul(
                            av[h][0:65, :],
                            lhsT=vsb[:, t, L, :],
                            rhs=ex[:, h * QCH:(h + 1) * QCH],
                            start=(t == 0), stop=(t == T - 1),
                        )

                def emit_out(c, q0, av):
                    for h in range(2):
                        L = c * 2 + h
                        oav = sbo.tile([65, QCH], dt.bfloat16, tag=f"oav{h}",
                                       name=f"oav{h}")
                        nc.vector.tensor_copy(oav, av[h][0:65, :])
                        nc.sync.dma_start(
                            av_d[L * 65:L * 65 + 65, q0 * QCH:(q0 + 1) * QCH],
                            oav)

                # ---- side-task queues: projections interleaved into the
                # attention stream (they fill PE stalls during exp waits) ----
                side = {c: deque() for c in range(4)}
                side[0].extend([(emit_vproj, (t,)) for t in range(2, T)])
                side[0].extend([(emit_qproj, (0, q0)) for q0 in (1, 2, 3)])
                for c in range(1, 4):
                    side[c].extend([(emit_kproj, (c, s, w)) for (s, w) in kcols])
                    side[c].extend([(emit_qproj, (c, q0)) for q0 in range(NQ)])

                # prefix: minimum projections for the first attention items
                for (s, w) in kcols:
                    emit_kproj(0, s, w)
                emit_qproj(0, 0)
                emit_vproj(0)
                if T > 1:
                    emit_vproj(1)
                nc.vector.tensor_copy(vsb[:, :, :, 64], vindst)

                # ---- software-pipelined attention stream ----
                stream = [(c, q0, t) for c in range(4) for q0 in range(NQ)
                          for t in range(T)]

                state = {}  # (c, q0) -> dict(av=, den=, ex={t: tile}, dfirst=)

                def do_X(item, eb, on_dve):
                    c, q0, t = item
                    st = state[(c, q0)]
                    ex = emit_X(eb, on_dve)
                    st["ex"][t] = ex

                def do_A(item):
                    c, q0, t = item
                    st = state[(c, q0)]
                    emit_A(c, t, st["ex"][t], st["av"])
                    st["ex"].pop(t, None)
                    if t == T - 1:
                        emit_out(c, q0, st["av"])
                        del state[(c, q0)]

                # software pipeline, per iteration j: E(j), X(j-1), A(j-2).
                # X(j-1) is emitted before E(j+1) (the next writer of its
                # psE buffer, bufs=2) so the WAR is tracked; every emitted
                # instruction's dependencies completed >= 1 full period ago,
                # so neither engine ever stalls mid-stream.
                pend_x = deque()
                pend_a = deque()
                for jdx, item in enumerate(stream):
                    c, q0, t = item
                    if t == 0:
                        state[(c, q0)] = {
                            "av": [psO.tile([128, QCH], dt.float32, tag=f"av{h}",
                                            name=f"av{h}") for h in range(2)],
                            "ex": {},
                        }
                    on_dve = jdx >= 16 and jdx % DVE_EXP_PERIOD == 0
                    eb = emit_E(c, q0, t)
                    if on_dve:
                        # DVE exp with zero lag: a full extra pipeline period
                        # of slack before its psE buffer is rewritten, so the
                        # (slower) Vector-engine exp never gates the stream
                        do_X(item, eb, True)
                    # side projection task right after the energy pair: it
                    # fills the PE queue without gating anything downstream,
                    # and runs 2+ iterations before its outputs are consumed
                    sq = None
                    if side[c]:
                        sq = side[c]
                    elif c + 1 < 4 and side[c + 1]:
                        sq = side[c + 1]
                    if sq:
                        fn, args = sq.popleft()
                        fn(*args)
                    pend_x.append((item, eb, on_dve))
                    if len(pend_x) >= 2:
                        it2, eb2, dv2 = pend_x.popleft()
                        if not dv2:
                            do_X(it2, eb2, False)
                        pend_a.append(it2)
                    if len(pend_a) >= 2:
                        do_A(pend_a.popleft())
                while pend_x:
                    it2, eb2, dv2 = pend_x.popleft()
                    if not dv2:
                        do_X(it2, eb2, False)
                    pend_a.append(it2)
                while pend_a:
                    do_A(pend_a.popleft())

    nc.compile()
    return nc


def _prepare(queries, keys, values, mask):
    """Host-side sharding: transpose, compact kv by mask, validity tiles."""
    m = np.asarray(mask).reshape(N, KLEN) != 0
    idx = [np.nonzero(m[n])[0] for n in range(N)]
    cnts = [len(i) for i in idx]
    T = max(1, (max(cnts) + 127) // 128)
    KC = 128 * T

    kT_full = np.ascontiguousarray(np.asarray(keys, np.float32)[0].T)
    vT_full = np.ascontiguousarray(np.asarray(values, np.float32)[0].T)
    q32 = np.asarray(queries, np.float32)

    def inter(x):
        # [512, cols] -> [128, 4, cols] partition-major interleave
        return np.ascontiguousarray(
            x.reshape(4, 128, -1).transpose(1, 0, 2)).astype(BF16)

    qT_n, kT_n, vT_n, vind_n = [], [], [], []
    for n in range(N):
        kt = np.zeros((KVDIM, KC), np.float32)
        vt = np.zeros((KVDIM, KC), np.float32)
        kt[:, :cnts[n]] = kT_full[:, idx[n]]
        vt[:, :cnts[n]] = vT_full[:, idx[n]]
        ind = (np.arange(KC) < cnts[n]).astype(np.float32)
        indT = ind.reshape(T, 128).T                       # [128, T]
        vind_n.append(np.ascontiguousarray(
            np.repeat(indT[:, :, None], 8, axis=2).reshape(128, T * 8)
        ).astype(BF16))
        kT_n.append(inter(kt))
        vT_n.append(inter(vt))
        qT_n.append(inter(np.ascontiguousarray(q32[n].T)))
    return T, qT_n, kT_n, vT_n, vind_n


def kernel(queries, keys, values, mask, Wq, Wk, Wv, _trace=False):
    global last_exec_time_ns, last_results
    T, qT_n, kT_n, vT_n, vind_n = _prepare(queries, keys, values, mask)

    w_g = {}
    for nm, W in (("wq", Wq), ("wk", Wk), ("wv", Wv)):
        W = np.asarray(W, np.float32)
        w_g[nm] = [np.ascontiguousarray(
            W[:, g * 512:(g + 1) * 512].reshape(4, 128, 512).transpose(1, 0, 2)
        ).astype(BF16) for g in range(2)]

    nc = _cache.get(T)
    if nc is None:
        nc = _cache.setdefault(T, _build(T))

    in_maps = []
    for core in range(N_CORES):
        n, g = core // 2, core % 2
        in_maps.append({
            "qt": qT_n[n], "kt": kT_n[n], "vt": vT_n[n],
            "wq": w_g["wq"][g], "wk": w_g["wk"][g], "wv": w_g["wv"][g],
            "vind": vind_n[n],
        })

    res = run_bass_kernel_spmd(nc, in_maps, core_ids=list(range(N_CORES)),
                               trace=bool(_trace))
    last_exec_time_ns = res.exec_time_ns
    last_results = res

    full = np.empty((N, QLEN, EMBED), np.float32)
    for core in range(N_CORES):
        n, g = core // 2, core % 2
        o = np.asarray(res.results[core]["av"], dtype=np.float32)
        o = o.reshape(8, 65, QLEN)                       # [L, d|denom, q]
        vals = o[:, :64, :] / o[:, 64:65, :]             # [8, 64, QLEN]
        full[n, :, g * 512:(g + 1) * 512] = (
            vals.transpose(2, 0, 1).reshape(QLEN, 512)
        )
    return full
